# revision 52
# baseline (speedup 1.0000x reference)
"""Trainium2 Bass kernel for a pre-RMSNorm attention+FFN transformer block.

Problem: x (2, 1024, 4096) fp32, channel-major (B, C, T).
  h = x^T; h += Attn(RMSNorm(h)); h += FFN(RMSNorm(h)); return h^T.

Sharding: 8 cores = 2 batches x 4 query-token chunks of 1024.  Each core
computes K/V for its batch's own 1024-token chunk, AllGathers K then V
across the 4-core replica group (one collective each; the cost model
charges a flat ~15us per collective and serializes them, so fewer,
earlier collectives win), then runs attention + Wo + FFN for its chunk.

All big matmuls run in fp8e4 with DoubleRow perf mode.  Weights are
prescaled on the host (x32 for Wq/Wk/Wv, x64 for Wo/W1/W2) so fp8's
normal range is used; scales fold back via activation scale factors and
fused scalar_tensor_tensor residual adds.  The softmax exp is split
between the Act engine (exact exp) and a DVE+Pool Schraudolph bit-trick
pipe so all three elementwise engines run hot; the denominator comes
from a DoubleRow matmul against a constant tile.  The residual path is
bf16 x + f32 accumulation.  DMAs are merged into few large strided
transfers (shared HWDGE serializes per-DMA overhead).
"""

import numpy as np
import ml_dtypes

import concourse.bass as bass
import concourse.mybir as mybir
import concourse.tile as tile
from concourse import bacc
from concourse.bass_utils import run_bass_kernel_spmd

F32 = mybir.dt.float32
BF16 = mybir.dt.bfloat16
FP8 = mybir.dt.float8e4
I32 = mybir.dt.int32
AF = mybir.ActivationFunctionType
ALU = mybir.AluOpType
DRW = mybir.MatmulPerfMode.DoubleRow

B = 2
C = 1024
T = 4096
TQ = 1024          # query-token chunk per core
H = 4
DH = 256
FF = 1536
P = 128
NT = 512           # moving-operand / PSUM tile width
CT = C // P        # 8 channel tiles
TQT = TQ // NT     # 2 chunk token tiles
DB = C // P        # 8 output-channel blocks for q/k/v/o
FFB = FF // P      # 12 ff blocks
TJ = T // P        # 32 key-token blocks
JP = TJ // 2       # 16 key-block pairs

WS_QKV = 32.0      # host prescale on Wq/Wk/Wv
WS = 64.0          # host prescale on Wo/W1/W2
OSC = 16.0         # scale of oT relative to true attention output
ONES_DEN = WS_QKV / OSC              # memset value for the denominator matmul
EXP_SCALE = (DH ** -0.5) / (WS_QKV * WS_QKV)
SCH_A = 12102203.161561485           # 2^23 / ln 2
SCH_B = 127.0 * (1 << 23) - 366000.0
SCH_SET = {2, 5, 8, 10, 12, 14}  # pairs routed to the DVE+Pool exp pipe

_CACHE = {}


def _build():
    nc = bacc.Bacc()
    xqb = nc.dram_tensor("xqb", [C, TQ], BF16, kind="ExternalInput")
    wq = nc.dram_tensor("wq", [C, C], FP8, kind="ExternalInput")
    wk = nc.dram_tensor("wk", [C, C], FP8, kind="ExternalInput")
    wv = nc.dram_tensor("wv", [C, C], FP8, kind="ExternalInput")
    wo = nc.dram_tensor("wo", [C, C], FP8, kind="ExternalInput")
    w1 = nc.dram_tensor("w1", [C, FF], FP8, kind="ExternalInput")
    w2 = nc.dram_tensor("w2", [FF, C], FP8, kind="ExternalInput")
    out = nc.dram_tensor("out", [C, TQ], F32, kind="ExternalOutput")

    RG = [[0, 1, 2, 3], [4, 5, 6, 7]]

    with tile.TileContext(nc) as tc:
        cpool_cm = tc.tile_pool(name="const", bufs=1)
        cpool = cpool_cm.__enter__()
        ones8 = cpool.tile([P, 2, P], FP8, tag="ones8", name="ones8")
        nc.vector.memset(ones8[:], 1.0)
        ones_b = cpool.tile([P, P], BF16, tag="ones_b", name="ones_b")
        nc.vector.memset(ones_b[:], 1.0)
        ones_d = cpool.tile([P, 2, P], FP8, tag="ones_d", name="ones_d")
        nc.vector.memset(ones_d[:], ONES_DEN)
        eps_t = cpool.tile([P, 1], F32, tag="eps", name="eps_t")
        nc.vector.memset(eps_t[:], 1e-8)

        dram_cm = tc.tile_pool(name="dram", bufs=1, space="DRAM")
        dp = dram_cm.__enter__()
        kl_d = dp.tile([C, TQ], FP8, tag="kl_d", name="kl_d")
        vl_d = dp.tile([TQ, C], FP8, tag="vl_d", name="vl_d")
        kg_d = dp.tile([4 * C, TQ], FP8, tag="kg_d", name="kg_d")
        vg_d = dp.tile([4 * TQ, C], FP8, tag="vg_d", name="vg_d")

        # ---- persistent SBUF (right side) ----
        qo_cm = tc.tile_pool(name="qopool", bufs=1, side="right")
        qop = qo_cm.__enter__()
        qT3 = qop.tile([P, DB, TQ], FP8, tag="qT", name="qT3")          # 8KB
        oT3 = qT3  # o^T reuses q^T: each (head, ti) slice is dead after scores

        xb_cm = tc.tile_pool(name="xbpool", bufs=1, side="right")
        xbp = xb_cm.__enter__()
        xbT = xbp.tile([P, CT, TQ], BF16, tag="xbT", name="xbT")        # 16KB

        pbA_cm = tc.tile_pool(name="pbA", bufs=1, side="right")
        pbA = pbA_cm.__enter__()
        wq3 = pbA.tile([P, CT, C], FP8, tag="wq3", name="wq3")
        wk3 = pbA.tile([P, CT, C], FP8, tag="wk3", name="wk3")
        wv3 = pbA.tile([P, CT, C], FP8, tag="wv3", name="wv3")
        aT3 = pbA.tile([P, CT, TQ], FP8, tag="aT3", name="aT3")

        # ---- input + weight DMAs (merged, ordered by need) ----
        xqb_r = xqb[:, :].rearrange("(c p) t -> p c t", p=P)
        nc.sync.dma_start(xbT[:, 0:CT // 2, 0:NT], xqb_r[:, 0:CT // 2, 0:NT])
        nc.sync.dma_start(xbT[:, CT // 2:CT, 0:NT], xqb_r[:, CT // 2:CT, 0:NT])
        nc.sync.dma_start(xbT[:, 0:CT // 2, NT:TQ], xqb_r[:, 0:CT // 2, NT:TQ])
        nc.sync.dma_start(xbT[:, CT // 2:CT, NT:TQ], xqb_r[:, CT // 2:CT, NT:TQ])
        nc.sync.dma_start(wk3[:, :, :],
                          wk[:, :].rearrange("(c p) n -> p c n", p=P))
        nc.sync.dma_start(wv3[:, :, :],
                          wv[:, :].rearrange("(c p) n -> p c n", p=P))

        pbN_cm = tc.tile_pool(name="pbN", bufs=1)
        pbN = pbN_cm.__enter__()
        pbps_cm = tc.tile_pool(name="pb_ps", bufs=1, space="PSUM")
        pbps = pbps_cm.__enter__()

        # ---- chunk rmsnorm -> aT3 (fp8) ----
        for t2 in range(TQT):
            x_sl = xbT[:, :, t2 * NT:(t2 + 1) * NT]
            sq3 = pbN.tile([P, CT, NT], BF16, tag="sqb", bufs=2, name="sqb")
            for cp_ in range(CT // 2):
                eng = nc.vector if cp_ % 2 == 0 else nc.gpsimd
                eng.tensor_mul(sq3[:, 2 * cp_:2 * cp_ + 2, :],
                               x_sl[:, 2 * cp_:2 * cp_ + 2, :],
                               x_sl[:, 2 * cp_:2 * cp_ + 2, :])
            ss = pbps.tile([P, NT], F32, tag="ssb", bufs=2, name="ssb")
            for ci in range(CT):
                nc.tensor.matmul(ss[:], ones_b[:], sq3[:, ci, :],
                                 start=(ci == 0), stop=(ci == CT - 1))
            sqt = pbN.tile([P, NT], F32, tag="sqtb", bufs=2, name="sqtb")
            nc.scalar.activation(sqt[:], ss[:], AF.Sqrt, scale=1.0 / C, bias=eps_t[:])
            rn = pbN.tile([P, NT], F32, tag="rnb", bufs=2, name="rnb")
            nc.vector.reciprocal(rn[:], sqt[:])
            for ci in range(CT):
                eng = nc.gpsimd if ci % 2 == 0 else nc.vector
                eng.tensor_mul(aT3[:, ci, t2 * NT:(t2 + 1) * NT],
                               x_sl[:, ci, :], rn[:])

        # ---- K/V staging in SBUF, 2 store DMAs, 2 collectives ----
        kvs_cm = tc.tile_pool(name="kvs", bufs=1)
        kvs = kvs_cm.__enter__()
        k8 = kvs.tile([P, DB, TQ], FP8, tag="k8", name="k8")            # 8KB
        v8 = kvs.tile([P, TQ // P, C], FP8, tag="v8", name="v8")        # 8KB

        cp_engs = [nc.scalar, nc.vector, nc.scalar, nc.vector]

        for dp_ in range(DB // 2):
            for t2 in range(TQT):
                pk = pbps.tile([P, 2, NT], F32, tag="pp", bufs=3, name="pk")
                for half in range(2):
                    db = 2 * dp_ + half
                    for cp_ in range(CT // 2):
                        nc.tensor.matmul(
                            pk[:, half, :],
                            wk3[:, 2 * cp_:2 * cp_ + 2, db * P:(db + 1) * P],
                            aT3[:, 2 * cp_:2 * cp_ + 2, t2 * NT:(t2 + 1) * NT],
                            start=(cp_ == 0), stop=(cp_ == CT // 2 - 1),
                            perf_mode=DRW)
                nc.scalar.copy(
                    k8[:, 2 * dp_, t2 * NT:(t2 + 1) * NT], pk[:, 0, :])
                nc.vector.tensor_copy(
                    k8[:, 2 * dp_ + 1, t2 * NT:(t2 + 1) * NT], pk[:, 1, :])
            nc.sync.dma_start(
                kl_d[:, :].rearrange("(c p) t -> p c t", p=P)[:, 2 * dp_:2 * dp_ + 2, :],
                k8[:, 2 * dp_:2 * dp_ + 2, :])

        for jl in range(TQ // P):
            pv = pbps.tile([P, 2, NT], F32, tag="pp", bufs=3, name="pv")
            for hf in range(2):
                for cp_ in range(CT // 2):
                    nc.tensor.matmul(
                        pv[:, hf, :],
                        aT3[:, 2 * cp_:2 * cp_ + 2, jl * P:(jl + 1) * P],
                        wv3[:, 2 * cp_:2 * cp_ + 2, hf * NT:(hf + 1) * NT],
                        start=(cp_ == 0), stop=(cp_ == CT // 2 - 1),
                        perf_mode=DRW)
            eng = cp_engs[jl % 4]
            if eng is nc.scalar:
                nc.scalar.copy(v8[:, jl, :], pv[:])
            else:
                eng.tensor_copy(v8[:, jl, :], pv[:])
            if jl % 2 == 1:
                nc.sync.dma_start(
                    vl_d[:, :].rearrange("(j p) c -> p j c", p=P)[:, jl - 1:jl + 1, :],
                    v8[:, jl - 1:jl + 1, :])

        nc.gpsimd.collective_compute(
            "AllGather", mybir.AluOpType.bypass, replica_groups=RG,
            ins=[kl_d[:, :]], outs=[kg_d[:, :]])
        nc.gpsimd.collective_compute(
            "AllGather", mybir.AluOpType.bypass, replica_groups=RG,
            ins=[vl_d[:, :]], outs=[vg_d[:, :]])

        # ---- Q (overlaps the collectives) ----
        nc.sync.dma_start(wq3[:, :, :],
                          wq[:, :].rearrange("(c p) n -> p c n", p=P))
        for t2 in range(TQT):
            for dp_ in range(DB // 2):
                pq = pbps.tile([P, 2, NT], F32, tag="pp", bufs=3, name="pq")
                for half in range(2):
                    db = 2 * dp_ + half
                    for cp_ in range(CT // 2):
                        nc.tensor.matmul(
                            pq[:, half, :],
                            wq3[:, 2 * cp_:2 * cp_ + 2, db * P:(db + 1) * P],
                            aT3[:, 2 * cp_:2 * cp_ + 2, t2 * NT:(t2 + 1) * NT],
                            start=(cp_ == 0), stop=(cp_ == CT // 2 - 1),
                            perf_mode=DRW)
                nc.scalar.copy(
                    qT3[:, 2 * dp_:2 * dp_ + 2, t2 * NT:(t2 + 1) * NT], pq[:])

        kvs_cm.__exit__(None, None, None)
        pbN_cm.__exit__(None, None, None)
        pbps_cm.__exit__(None, None, None)
        pbA_cm.__exit__(None, None, None)

        # ---- more weights during the collective window ----
        hR_cm = tc.tile_pool(name="hpool", bufs=1, side="right")
        hRp = hR_cm.__enter__()
        hB = hRp.tile([P, CT, TQ], F32, tag="hB", name="hB")            # 32KB
        pe_cm = tc.tile_pool(name="pe", bufs=1, side="right")
        pep = pe_cm.__enter__()
        w13 = pep.tile([P, CT, FF], FP8, tag="w13", name="w13")         # 12KB
        nc.scalar.dma_start(w13[:, :, :],
                            w1[:, :].rearrange("(c p) n -> p c n", p=P))
        fB3 = pep.tile([P, CT, TQ], FP8, tag="fB3", name="fB3")         # 8KB
        wo_cm = tc.tile_pool(name="wopool", bufs=1, side="right")
        wop = wo_cm.__enter__()
        wo3 = wop.tile([P, CT, C], FP8, tag="wo3", name="wo3")
        nc.scalar.dma_start(wo3[:, :, :],
                            wo[:, :].rearrange("(c p) n -> p c n", p=P))

        # ---- gathered K/V reload: K first (scores need it), then V ----
        kT_cm = tc.tile_pool(name="kTpool", bufs=1)
        kTp = kT_cm.__enter__()
        kT3 = kTp.tile([P, DB, T], FP8, tag="kT", name="kT3")           # 32KB
        vB_cm = tc.tile_pool(name="vBpool", bufs=1)
        vBp = vB_cm.__enter__()
        vB3 = vBp.tile([P, TJ, C], FP8, tag="vB", name="vB3")           # 32KB

        kg_r = kg_d[:, :].rearrange("(r c p) t -> p r c t", p=P, r=4)
        for hp in range(H):
            for r in range(4):
                nc.sync.dma_start(
                    kT3[:, 2 * hp:2 * hp + 2, r * TQ:(r + 1) * TQ],
                    kg_r[:, r, 2 * hp:2 * hp + 2, :])
        vg_r = vg_d[:, :].rearrange("(r j p) c -> p r j c", p=P, r=4)
        for r in range(4):
            nc.sync.dma_start(vB3[:, r * (TQ // P):(r + 1) * (TQ // P), 0:NT],
                              vg_r[:, r, :, 0:NT])
        for r in range(4):
            nc.sync.dma_start(vB3[:, r * (TQ // P):(r + 1) * (TQ // P), NT:C],
                              vg_r[:, r, :, NT:C])

        # ---------------- attention (+ interleaved Wo/residual) ----------------
        pc_cm = tc.tile_pool(name="pc", bufs=1)
        pcp = pc_cm.__enter__()
        pss_cm = tc.tile_pool(name="ps_s", bufs=2, space="PSUM")
        pss = pss_cm.__enter__()
        pso_cm = tc.tile_pool(name="ps_o", bufs=1, space="PSUM")
        pso = pso_cm.__enter__()
        for ti in range(TQT):
            for h in range(H):
                et3 = pcp.tile([P, TJ, NT], FP8, tag="exp", bufs=2, name="et3")
                q_sl = qT3[:, 2 * h:2 * h + 2, ti * NT:(ti + 1) * NT]
                HN = NT // 2
                for jp in range(JP):
                    for qh in range(2):
                        psc = pss.tile([P, 2, HN], F32, tag="s", bufs=4,
                                       name="psc")
                        for half in range(2):
                            tj = 2 * jp + half
                            nc.tensor.matmul(
                                psc[:, half, :],
                                kT3[:, 2 * h:2 * h + 2, tj * P:(tj + 1) * P],
                                q_sl[:, :, qh * HN:(qh + 1) * HN],
                                start=True, stop=True, perf_mode=DRW)
                        e_sl = et3[:, 2 * jp:2 * jp + 2, qh * HN:(qh + 1) * HN]
                        if jp in SCH_SET or (jp == 6 and qh == 1):
                            sch = pcp.tile([P, 2, HN], I32, tag="sch", bufs=8,
                                           name="sch")
                            nc.vector.tensor_scalar(sch[:], psc[:],
                                                    SCH_A * EXP_SCALE, SCH_B,
                                                    ALU.mult, ALU.add)
                            nc.gpsimd.tensor_copy(e_sl, sch[:].bitcast(F32))
                        else:
                            nc.scalar.activation(e_sl, psc[:], AF.Exp,
                                                 scale=EXP_SCALE)
                po0 = pso.tile([P, NT], F32, tag="po0", name="po0")
                po1 = pso.tile([P, NT], F32, tag="po1", name="po1")
                pr = pso.tile([P, NT], F32, tag="pr", name="pr")
                for jp in range(JP):
                    e_sl = et3[:, 2 * jp:2 * jp + 2, :]
                    st_, sp_ = (jp == 0), (jp == JP - 1)
                    nc.tensor.matmul(pr[:], ones_d[:], e_sl, start=st_, stop=sp_,
                                     perf_mode=DRW, skip_group_check=True)
                    nc.tensor.matmul(po0[:],
                                     vB3[:, 2 * jp:2 * jp + 2, h * DH: h * DH + P],
                                     e_sl, start=st_, stop=sp_,
                                     perf_mode=DRW, skip_group_check=True)
                    nc.tensor.matmul(po1[:],
                                     vB3[:, 2 * jp:2 * jp + 2, h * DH + P:(h + 1) * DH],
                                     e_sl, start=st_, stop=sp_,
                                     perf_mode=DRW, skip_group_check=True)
                rec = pcp.tile([P, NT], F32, tag="rec", bufs=1, name="rec")
                nc.vector.reciprocal(rec[:], pr[:])
                nc.vector.tensor_mul(oT3[:, 2 * h, ti * NT:(ti + 1) * NT],
                                     po0[:], rec[:])
                nc.vector.tensor_mul(oT3[:, 2 * h + 1, ti * NT:(ti + 1) * NT],
                                     po1[:], rec[:])
            # ---- Wo + residual for this token half ----
            t2 = ti
            for cb in range(CT):
                if t2 == TQT - 1:
                    ph_tag = ("ph", "po0", "po1")[cb % 3]
                else:
                    ph_tag = "ph"
                ph = pso.tile([P, NT], F32, tag=ph_tag, bufs=1, name="ph")
                for cp_ in range(CT // 2):
                    nc.tensor.matmul(
                        ph[:],
                        wo3[:, 2 * cp_:2 * cp_ + 2, cb * P:(cb + 1) * P],
                        oT3[:, 2 * cp_:2 * cp_ + 2, t2 * NT:(t2 + 1) * NT],
                        start=(cp_ == 0), stop=(cp_ == CT // 2 - 1),
                        perf_mode=DRW)
                nc.vector.scalar_tensor_tensor(
                    hB[:, cb, t2 * NT:(t2 + 1) * NT], ph[:],
                    1.0 / (OSC * WS), xbT[:, cb, t2 * NT:(t2 + 1) * NT],
                    ALU.mult, ALU.add)
        pso_cm.__exit__(None, None, None)
        pss_cm.__exit__(None, None, None)
        pc_cm.__exit__(None, None, None)
        vB_cm.__exit__(None, None, None)
        kT_cm.__exit__(None, None, None)
        wo_cm.__exit__(None, None, None)

        # w23 loads into the space freed by the attention pools
        pf_cm = tc.tile_pool(name="pf", bufs=1)
        pfp = pf_cm.__enter__()
        w23 = pfp.tile([P, FFB, C], FP8, tag="w23", name="w23")         # 12KB
        gB3 = pfp.tile([P, FFB, TQ], FP8, tag="gB3", name="gB3")        # 12KB
        nc.sync.dma_start(w23[:, :, :],
                          w2[:, :].rearrange("(f p) n -> p f n", p=P))

        # ---------------- FFN ----------------
        peps_cm = tc.tile_pool(name="pe_ps", bufs=2, space="PSUM")
        peps = peps_cm.__enter__()
        for t2 in range(TQT):
            sq3 = pep.tile([P, CT, NT], FP8, tag="sqe", bufs=1, name="sqe")
            for cp_ in range(CT // 2):
                eng = nc.gpsimd if cp_ != 3 else nc.vector
                eng.tensor_mul(sq3[:, 2 * cp_:2 * cp_ + 2, :],
                               hB[:, 2 * cp_:2 * cp_ + 2, t2 * NT:(t2 + 1) * NT],
                               hB[:, 2 * cp_:2 * cp_ + 2, t2 * NT:(t2 + 1) * NT])
            ss = peps.tile([P, NT], F32, tag="sse", bufs=1, name="sse")
            for cp_ in range(CT // 2):
                nc.tensor.matmul(ss[:], ones8[:], sq3[:, 2 * cp_:2 * cp_ + 2, :],
                                 start=(cp_ == 0), stop=(cp_ == CT // 2 - 1),
                                 perf_mode=DRW)
            sqt = pep.tile([P, NT], F32, tag="sqte", bufs=1, name="sqte")
            nc.scalar.activation(sqt[:], ss[:], AF.Sqrt, scale=1.0 / C, bias=eps_t[:])
            rn = pep.tile([P, NT], F32, tag="rne", bufs=1, name="rne")
            nc.vector.reciprocal(rn[:], sqt[:])
            for ci in range(CT):
                eng = nc.gpsimd if ci % 4 != 3 else nc.vector
                eng.tensor_mul(fB3[:, ci, t2 * NT:(t2 + 1) * NT],
                               hB[:, ci, t2 * NT:(t2 + 1) * NT], rn[:])
        for t2 in range(TQT):
            for fp_ in range(FFB // 2):
                pu = peps.tile([P, 2, NT], F32, tag="pu", bufs=2, name="pu")
                for half in range(2):
                    fb = 2 * fp_ + half
                    for cp_ in range(CT // 2):
                        nc.tensor.matmul(
                            pu[:, half, :],
                            w13[:, 2 * cp_:2 * cp_ + 2, fb * P:(fb + 1) * P],
                            fB3[:, 2 * cp_:2 * cp_ + 2, t2 * NT:(t2 + 1) * NT],
                            start=(cp_ == 0), stop=(cp_ == CT // 2 - 1),
                            perf_mode=DRW)
                nc.scalar.activation(
                    gB3[:, 2 * fp_:2 * fp_ + 2, t2 * NT:(t2 + 1) * NT],
                    pu[:], AF.Gelu, scale=1.0 / WS)
        for t2 in range(TQT):
            yB = pfp.tile([P, CT, NT], F32, tag="yB", bufs=2, name="yB")
            for cb in range(CT):
                py = peps.tile([P, NT], F32, tag="py", bufs=3, name="py")
                for fp_ in range(FFB // 2):
                    nc.tensor.matmul(
                        py[:],
                        w23[:, 2 * fp_:2 * fp_ + 2, cb * P:(cb + 1) * P],
                        gB3[:, 2 * fp_:2 * fp_ + 2, t2 * NT:(t2 + 1) * NT],
                        start=(fp_ == 0), stop=(fp_ == FFB // 2 - 1),
                        perf_mode=DRW)
                nc.vector.scalar_tensor_tensor(
                    yB[:, cb, :], py[:], 1.0 / WS,
                    hB[:, cb, t2 * NT:(t2 + 1) * NT], ALU.mult, ALU.add)
            out_r = out[:, :].rearrange("(c p) t -> p c t", p=P)
            for cq_ in range(CT):
                nc.sync.dma_start(
                    out_r[:, cq_:cq_ + 1, t2 * NT:(t2 + 1) * NT],
                    yB[:, cq_:cq_ + 1, :])
        peps_cm.__exit__(None, None, None)
        pf_cm.__exit__(None, None, None)
        pe_cm.__exit__(None, None, None)
        hR_cm.__exit__(None, None, None)
        xb_cm.__exit__(None, None, None)
        qo_cm.__exit__(None, None, None)
        dram_cm.__exit__(None, None, None)
        cpool_cm.__exit__(None, None, None)

        sched_state, snap = tc.schedule_and_allocate()
        _CACHE["predicted_ns"] = snap.time if snap is not None else None
        try:
            _CACHE["dispatch_ns"] = sched_state.get_inst_dispatch_ns()
        except Exception:
            _CACHE["dispatch_ns"] = None

    nc.finalize()
    return nc


def get_nc():
    if "nc" not in _CACHE:
        _CACHE["nc"] = _build()
    return _CACHE["nc"]


def _prep_inputs(inputs):
    f8 = ml_dtypes.float8_e4m3
    x = np.asarray(inputs["x"], dtype=np.float32)
    g_attn = np.asarray(inputs["g_attn"], np.float32)
    g_ff = np.asarray(inputs["g_ff"], np.float32)
    wq8 = (g_attn[:, None] * np.asarray(inputs["Wq"], np.float32) * WS_QKV).astype(f8)
    wk8 = (g_attn[:, None] * np.asarray(inputs["Wk"], np.float32) * WS_QKV).astype(f8)
    wv8 = (g_attn[:, None] * np.asarray(inputs["Wv"], np.float32) * WS_QKV).astype(f8)
    wo8 = (np.asarray(inputs["Wo"], np.float32) * WS).astype(f8)
    w18 = (g_ff[:, None] * np.asarray(inputs["W1"], np.float32) * WS).astype(f8)
    w28 = (np.asarray(inputs["W2"], np.float32) * WS).astype(f8)
    in_maps = []
    for core in range(8):
        b, cq = divmod(core, 4)
        xc = np.ascontiguousarray(x[b][:, cq * TQ:(cq + 1) * TQ])
        in_maps.append({
            "xqb": xc.astype(ml_dtypes.bfloat16),
            "wq": wq8, "wk": wk8, "wv": wv8, "wo": wo8, "w1": w18, "w2": w28,
        })
    return in_maps


def run(inputs, **kwargs):
    nc = get_nc()
    in_maps = _prep_inputs(inputs)
    res = run_bass_kernel_spmd(nc, in_maps, core_ids=list(range(8)), **kwargs)
    out = np.empty((B, C, T), np.float32)
    for core in range(8):
        b, cq = divmod(core, 4)
        out[b][:, cq * TQ:(cq + 1) * TQ] = res.results[core]["out"]
    return out, res


def kernel(**inputs) -> np.ndarray:
    out, _ = run(inputs)
    return out


# revision 53
# speedup vs baseline: 1.0015x; 1.0015x over previous
"""Trainium2 Bass kernel for a pre-RMSNorm attention+FFN transformer block.

Problem: x (2, 1024, 4096) fp32, channel-major (B, C, T).
  h = x^T; h += Attn(RMSNorm(h)); h += FFN(RMSNorm(h)); return h^T.

Sharding: 8 cores = 2 batches x 4 query-token chunks of 1024.  Each core
computes K/V for its batch's own 1024-token chunk, AllGathers K then V
across the 4-core replica group (one collective each; the cost model
charges a flat ~15us per collective and serializes them, so fewer,
earlier collectives win), then runs attention + Wo + FFN for its chunk.

All big matmuls run in fp8e4 with DoubleRow perf mode.  Weights are
prescaled on the host (x32 for Wq/Wk/Wv, x64 for Wo/W1/W2) so fp8's
normal range is used; scales fold back via activation scale factors and
fused scalar_tensor_tensor residual adds.  The softmax exp is split
between the Act engine (exact exp) and a DVE+Pool Schraudolph bit-trick
pipe so all three elementwise engines run hot; the denominator comes
from a DoubleRow matmul against a constant tile.  The residual path is
bf16 x + f32 accumulation.  DMAs are merged into few large strided
transfers (shared HWDGE serializes per-DMA overhead).
"""

import numpy as np
import ml_dtypes

import concourse.bass as bass
import concourse.mybir as mybir
import concourse.tile as tile
from concourse import bacc
from concourse.bass_utils import run_bass_kernel_spmd

F32 = mybir.dt.float32
BF16 = mybir.dt.bfloat16
FP8 = mybir.dt.float8e4
I32 = mybir.dt.int32
AF = mybir.ActivationFunctionType
ALU = mybir.AluOpType
DRW = mybir.MatmulPerfMode.DoubleRow

B = 2
C = 1024
T = 4096
TQ = 1024          # query-token chunk per core
H = 4
DH = 256
FF = 1536
P = 128
NT = 512           # moving-operand / PSUM tile width
CT = C // P        # 8 channel tiles
TQT = TQ // NT     # 2 chunk token tiles
DB = C // P        # 8 output-channel blocks for q/k/v/o
FFB = FF // P      # 12 ff blocks
TJ = T // P        # 32 key-token blocks
JP = TJ // 2       # 16 key-block pairs

WS_QKV = 32.0      # host prescale on Wq/Wk/Wv
WS = 64.0          # host prescale on Wo/W1/W2
OSC = 16.0         # scale of oT relative to true attention output
ONES_DEN = WS_QKV / OSC              # memset value for the denominator matmul
EXP_SCALE = (DH ** -0.5) / (WS_QKV * WS_QKV)
SCH_A = 12102203.161561485           # 2^23 / ln 2
SCH_B = 127.0 * (1 << 23) - 366000.0
SCH_SET = {2, 5, 8, 10, 12, 14}  # pairs routed to the DVE+Pool exp pipe

_CACHE = {}


def _build():
    nc = bacc.Bacc()
    xqb = nc.dram_tensor("xqb", [C, TQ], BF16, kind="ExternalInput")
    wq = nc.dram_tensor("wq", [C, C], FP8, kind="ExternalInput")
    wk = nc.dram_tensor("wk", [C, C], FP8, kind="ExternalInput")
    wv = nc.dram_tensor("wv", [C, C], FP8, kind="ExternalInput")
    wo = nc.dram_tensor("wo", [C, C], FP8, kind="ExternalInput")
    w1 = nc.dram_tensor("w1", [C, FF], FP8, kind="ExternalInput")
    w2 = nc.dram_tensor("w2", [FF, C], FP8, kind="ExternalInput")
    out = nc.dram_tensor("out", [C, TQ], F32, kind="ExternalOutput")

    RG = [[0, 1, 2, 3], [4, 5, 6, 7]]

    with tile.TileContext(nc) as tc:
        cpool_cm = tc.tile_pool(name="const", bufs=1)
        cpool = cpool_cm.__enter__()
        ones8 = cpool.tile([P, 2, P], FP8, tag="ones8", name="ones8")
        nc.vector.memset(ones8[:], 1.0)
        ones_b = cpool.tile([P, P], BF16, tag="ones_b", name="ones_b")
        nc.vector.memset(ones_b[:], 1.0)
        ones_d = cpool.tile([P, 2, P], FP8, tag="ones_d", name="ones_d")
        nc.vector.memset(ones_d[:], ONES_DEN)
        eps_t = cpool.tile([P, 1], F32, tag="eps", name="eps_t")
        nc.vector.memset(eps_t[:], 1e-8)

        dram_cm = tc.tile_pool(name="dram", bufs=1, space="DRAM")
        dp = dram_cm.__enter__()
        kl_d = dp.tile([C, TQ], FP8, tag="kl_d", name="kl_d")
        vl_d = dp.tile([TQ, C], FP8, tag="vl_d", name="vl_d")
        kg_d = dp.tile([4 * C, TQ], FP8, tag="kg_d", name="kg_d")
        vg_d = dp.tile([4 * TQ, C], FP8, tag="vg_d", name="vg_d")

        # ---- persistent SBUF (right side) ----
        qo_cm = tc.tile_pool(name="qopool", bufs=1, side="right")
        qop = qo_cm.__enter__()
        qT3 = qop.tile([P, DB, TQ], FP8, tag="qT", name="qT3")          # 8KB
        oT3 = qT3  # o^T reuses q^T: each (head, ti) slice is dead after scores

        xb_cm = tc.tile_pool(name="xbpool", bufs=1, side="right")
        xbp = xb_cm.__enter__()
        xbT = xbp.tile([P, CT, TQ], BF16, tag="xbT", name="xbT")        # 16KB

        pbA_cm = tc.tile_pool(name="pbA", bufs=1, side="right")
        pbA = pbA_cm.__enter__()
        wq3 = pbA.tile([P, CT, C], FP8, tag="wq3", name="wq3")
        wk3 = pbA.tile([P, CT, C], FP8, tag="wk3", name="wk3")
        wv3 = pbA.tile([P, CT, C], FP8, tag="wv3", name="wv3")
        aT3 = pbA.tile([P, CT, TQ], FP8, tag="aT3", name="aT3")

        # ---- input + weight DMAs (merged, ordered by need) ----
        xqb_r = xqb[:, :].rearrange("(c p) t -> p c t", p=P)
        nc.sync.dma_start(xbT[:, 0:CT // 2, 0:NT], xqb_r[:, 0:CT // 2, 0:NT])
        nc.sync.dma_start(xbT[:, CT // 2:CT, 0:NT], xqb_r[:, CT // 2:CT, 0:NT])
        nc.sync.dma_start(xbT[:, 0:CT // 2, NT:TQ], xqb_r[:, 0:CT // 2, NT:TQ])
        nc.sync.dma_start(xbT[:, CT // 2:CT, NT:TQ], xqb_r[:, CT // 2:CT, NT:TQ])
        nc.sync.dma_start(wk3[:, :, :],
                          wk[:, :].rearrange("(c p) n -> p c n", p=P))
        nc.sync.dma_start(wv3[:, :, :],
                          wv[:, :].rearrange("(c p) n -> p c n", p=P))

        pbN_cm = tc.tile_pool(name="pbN", bufs=1)
        pbN = pbN_cm.__enter__()
        pbps_cm = tc.tile_pool(name="pb_ps", bufs=1, space="PSUM")
        pbps = pbps_cm.__enter__()

        # ---- chunk rmsnorm -> aT3 (fp8) ----
        for t2 in range(TQT):
            x_sl = xbT[:, :, t2 * NT:(t2 + 1) * NT]
            sq3 = pbN.tile([P, CT, NT], BF16, tag="sqb", bufs=2, name="sqb")
            for cp_ in range(CT // 2):
                eng = nc.vector if cp_ % 2 == 0 else nc.gpsimd
                eng.tensor_mul(sq3[:, 2 * cp_:2 * cp_ + 2, :],
                               x_sl[:, 2 * cp_:2 * cp_ + 2, :],
                               x_sl[:, 2 * cp_:2 * cp_ + 2, :])
            ss = pbps.tile([P, NT], F32, tag="ssb", bufs=2, name="ssb")
            for ci in range(CT):
                nc.tensor.matmul(ss[:], ones_b[:], sq3[:, ci, :],
                                 start=(ci == 0), stop=(ci == CT - 1))
            sqt = pbN.tile([P, NT], F32, tag="sqtb", bufs=2, name="sqtb")
            nc.scalar.activation(sqt[:], ss[:], AF.Sqrt, scale=1.0 / C, bias=eps_t[:])
            rn = pbN.tile([P, NT], F32, tag="rnb", bufs=2, name="rnb")
            nc.vector.reciprocal(rn[:], sqt[:])
            for ci in range(CT):
                eng = nc.gpsimd if ci % 2 == 0 else nc.vector
                eng.tensor_mul(aT3[:, ci, t2 * NT:(t2 + 1) * NT],
                               x_sl[:, ci, :], rn[:])

        # ---- K/V staging in SBUF, 2 store DMAs, 2 collectives ----
        kvs_cm = tc.tile_pool(name="kvs", bufs=1)
        kvs = kvs_cm.__enter__()
        k8 = kvs.tile([P, DB, TQ], FP8, tag="k8", name="k8")            # 8KB
        v8 = kvs.tile([P, TQ // P, C], FP8, tag="v8", name="v8")        # 8KB

        cp_engs = [nc.scalar, nc.vector, nc.scalar, nc.vector]

        for dp_ in range(DB // 2):
            for t2 in range(TQT):
                pk = pbps.tile([P, 2, NT], F32, tag="pp", bufs=3, name="pk")
                for half in range(2):
                    db = 2 * dp_ + half
                    for cp_ in range(CT // 2):
                        nc.tensor.matmul(
                            pk[:, half, :],
                            wk3[:, 2 * cp_:2 * cp_ + 2, db * P:(db + 1) * P],
                            aT3[:, 2 * cp_:2 * cp_ + 2, t2 * NT:(t2 + 1) * NT],
                            start=(cp_ == 0), stop=(cp_ == CT // 2 - 1),
                            perf_mode=DRW)
                nc.scalar.copy(
                    k8[:, 2 * dp_, t2 * NT:(t2 + 1) * NT], pk[:, 0, :])
                nc.vector.tensor_copy(
                    k8[:, 2 * dp_ + 1, t2 * NT:(t2 + 1) * NT], pk[:, 1, :])
                nc.sync.dma_start(
                    kl_d[:, :].rearrange("(c p) t -> p c t", p=P)
                    [:, 2 * dp_:2 * dp_ + 2, t2 * NT:(t2 + 1) * NT],
                    k8[:, 2 * dp_:2 * dp_ + 2, t2 * NT:(t2 + 1) * NT])

        for jl in range(TQ // P):
            pv = pbps.tile([P, 2, NT], F32, tag="pp", bufs=3, name="pv")
            for hf in range(2):
                for cp_ in range(CT // 2):
                    nc.tensor.matmul(
                        pv[:, hf, :],
                        aT3[:, 2 * cp_:2 * cp_ + 2, jl * P:(jl + 1) * P],
                        wv3[:, 2 * cp_:2 * cp_ + 2, hf * NT:(hf + 1) * NT],
                        start=(cp_ == 0), stop=(cp_ == CT // 2 - 1),
                        perf_mode=DRW)
            eng = cp_engs[jl % 4]
            if eng is nc.scalar:
                nc.scalar.copy(v8[:, jl, :], pv[:])
            else:
                eng.tensor_copy(v8[:, jl, :], pv[:])
            if jl % 2 == 1:
                nc.sync.dma_start(
                    vl_d[:, :].rearrange("(j p) c -> p j c", p=P)[:, jl - 1:jl + 1, :],
                    v8[:, jl - 1:jl + 1, :])

        nc.gpsimd.collective_compute(
            "AllGather", mybir.AluOpType.bypass, replica_groups=RG,
            ins=[kl_d[:, :]], outs=[kg_d[:, :]])
        nc.gpsimd.collective_compute(
            "AllGather", mybir.AluOpType.bypass, replica_groups=RG,
            ins=[vl_d[:, :]], outs=[vg_d[:, :]])

        # ---- Q (overlaps the collectives) ----
        nc.sync.dma_start(wq3[:, :, :],
                          wq[:, :].rearrange("(c p) n -> p c n", p=P))
        for t2 in range(TQT):
            for dp_ in range(DB // 2):
                pq = pbps.tile([P, 2, NT], F32, tag="pp", bufs=3, name="pq")
                for half in range(2):
                    db = 2 * dp_ + half
                    for cp_ in range(CT // 2):
                        nc.tensor.matmul(
                            pq[:, half, :],
                            wq3[:, 2 * cp_:2 * cp_ + 2, db * P:(db + 1) * P],
                            aT3[:, 2 * cp_:2 * cp_ + 2, t2 * NT:(t2 + 1) * NT],
                            start=(cp_ == 0), stop=(cp_ == CT // 2 - 1),
                            perf_mode=DRW)
                nc.scalar.copy(
                    qT3[:, 2 * dp_:2 * dp_ + 2, t2 * NT:(t2 + 1) * NT], pq[:])

        kvs_cm.__exit__(None, None, None)
        pbN_cm.__exit__(None, None, None)
        pbps_cm.__exit__(None, None, None)
        pbA_cm.__exit__(None, None, None)

        # ---- more weights during the collective window ----
        hR_cm = tc.tile_pool(name="hpool", bufs=1, side="right")
        hRp = hR_cm.__enter__()
        hB = hRp.tile([P, CT, TQ], F32, tag="hB", name="hB")            # 32KB
        pe_cm = tc.tile_pool(name="pe", bufs=1, side="right")
        pep = pe_cm.__enter__()
        w13 = pep.tile([P, CT, FF], FP8, tag="w13", name="w13")         # 12KB
        nc.scalar.dma_start(w13[:, :, :],
                            w1[:, :].rearrange("(c p) n -> p c n", p=P))
        fB3 = pep.tile([P, CT, TQ], FP8, tag="fB3", name="fB3")         # 8KB
        wo_cm = tc.tile_pool(name="wopool", bufs=1, side="right")
        wop = wo_cm.__enter__()
        wo3 = wop.tile([P, CT, C], FP8, tag="wo3", name="wo3")
        nc.scalar.dma_start(wo3[:, :, :],
                            wo[:, :].rearrange("(c p) n -> p c n", p=P))

        # ---- gathered K/V reload: K first (scores need it), then V ----
        kT_cm = tc.tile_pool(name="kTpool", bufs=1)
        kTp = kT_cm.__enter__()
        kT3 = kTp.tile([P, DB, T], FP8, tag="kT", name="kT3")           # 32KB
        vB_cm = tc.tile_pool(name="vBpool", bufs=1)
        vBp = vB_cm.__enter__()
        vB3 = vBp.tile([P, TJ, C], FP8, tag="vB", name="vB3")           # 32KB

        kg_r = kg_d[:, :].rearrange("(r c p) t -> p r c t", p=P, r=4)
        for hp in range(H):
            for r in range(4):
                nc.sync.dma_start(
                    kT3[:, 2 * hp:2 * hp + 2, r * TQ:(r + 1) * TQ],
                    kg_r[:, r, 2 * hp:2 * hp + 2, :])
        vg_r = vg_d[:, :].rearrange("(r j p) c -> p r j c", p=P, r=4)
        for r in range(4):
            nc.sync.dma_start(vB3[:, r * (TQ // P):(r + 1) * (TQ // P), 0:NT],
                              vg_r[:, r, :, 0:NT])
        for r in range(4):
            nc.sync.dma_start(vB3[:, r * (TQ // P):(r + 1) * (TQ // P), NT:C],
                              vg_r[:, r, :, NT:C])

        # ---------------- attention (+ interleaved Wo/residual) ----------------
        pc_cm = tc.tile_pool(name="pc", bufs=1)
        pcp = pc_cm.__enter__()
        pss_cm = tc.tile_pool(name="ps_s", bufs=2, space="PSUM")
        pss = pss_cm.__enter__()
        pso_cm = tc.tile_pool(name="ps_o", bufs=1, space="PSUM")
        pso = pso_cm.__enter__()
        for ti in range(TQT):
            for h in range(H):
                et3 = pcp.tile([P, TJ, NT], FP8, tag="exp", bufs=2, name="et3")
                q_sl = qT3[:, 2 * h:2 * h + 2, ti * NT:(ti + 1) * NT]
                HN = NT // 2
                for jp in range(JP):
                    for qh in range(2):
                        psc = pss.tile([P, 2, HN], F32, tag="s", bufs=4,
                                       name="psc")
                        for half in range(2):
                            tj = 2 * jp + half
                            nc.tensor.matmul(
                                psc[:, half, :],
                                kT3[:, 2 * h:2 * h + 2, tj * P:(tj + 1) * P],
                                q_sl[:, :, qh * HN:(qh + 1) * HN],
                                start=True, stop=True, perf_mode=DRW)
                        e_sl = et3[:, 2 * jp:2 * jp + 2, qh * HN:(qh + 1) * HN]
                        if jp in SCH_SET or (jp == 6 and qh == 1):
                            sch = pcp.tile([P, 2, HN], I32, tag="sch", bufs=8,
                                           name="sch")
                            nc.vector.tensor_scalar(sch[:], psc[:],
                                                    SCH_A * EXP_SCALE, SCH_B,
                                                    ALU.mult, ALU.add)
                            nc.gpsimd.tensor_copy(e_sl, sch[:].bitcast(F32))
                        else:
                            nc.scalar.activation(e_sl, psc[:], AF.Exp,
                                                 scale=EXP_SCALE)
                po0 = pso.tile([P, NT], F32, tag="po0", name="po0")
                po1 = pso.tile([P, NT], F32, tag="po1", name="po1")
                pr = pso.tile([P, NT], F32, tag="pr", name="pr")
                for jp in range(JP):
                    e_sl = et3[:, 2 * jp:2 * jp + 2, :]
                    st_, sp_ = (jp == 0), (jp == JP - 1)
                    nc.tensor.matmul(pr[:], ones_d[:], e_sl, start=st_, stop=sp_,
                                     perf_mode=DRW, skip_group_check=True)
                    nc.tensor.matmul(po0[:],
                                     vB3[:, 2 * jp:2 * jp + 2, h * DH: h * DH + P],
                                     e_sl, start=st_, stop=sp_,
                                     perf_mode=DRW, skip_group_check=True)
                    nc.tensor.matmul(po1[:],
                                     vB3[:, 2 * jp:2 * jp + 2, h * DH + P:(h + 1) * DH],
                                     e_sl, start=st_, stop=sp_,
                                     perf_mode=DRW, skip_group_check=True)
                rec = pcp.tile([P, NT], F32, tag="rec", bufs=1, name="rec")
                nc.vector.reciprocal(rec[:], pr[:])
                nc.vector.tensor_mul(oT3[:, 2 * h, ti * NT:(ti + 1) * NT],
                                     po0[:], rec[:])
                nc.vector.tensor_mul(oT3[:, 2 * h + 1, ti * NT:(ti + 1) * NT],
                                     po1[:], rec[:])
            # ---- Wo + residual for this token half ----
            t2 = ti
            for cb in range(CT):
                if t2 == TQT - 1:
                    ph_tag = ("ph", "po0", "po1")[cb % 3]
                else:
                    ph_tag = "ph"
                ph = pso.tile([P, NT], F32, tag=ph_tag, bufs=1, name="ph")
                for cp_ in range(CT // 2):
                    nc.tensor.matmul(
                        ph[:],
                        wo3[:, 2 * cp_:2 * cp_ + 2, cb * P:(cb + 1) * P],
                        oT3[:, 2 * cp_:2 * cp_ + 2, t2 * NT:(t2 + 1) * NT],
                        start=(cp_ == 0), stop=(cp_ == CT // 2 - 1),
                        perf_mode=DRW)
                nc.vector.scalar_tensor_tensor(
                    hB[:, cb, t2 * NT:(t2 + 1) * NT], ph[:],
                    1.0 / (OSC * WS), xbT[:, cb, t2 * NT:(t2 + 1) * NT],
                    ALU.mult, ALU.add)
        pso_cm.__exit__(None, None, None)
        pss_cm.__exit__(None, None, None)
        pc_cm.__exit__(None, None, None)
        vB_cm.__exit__(None, None, None)
        kT_cm.__exit__(None, None, None)
        wo_cm.__exit__(None, None, None)

        # w23 loads into the space freed by the attention pools
        pf_cm = tc.tile_pool(name="pf", bufs=1)
        pfp = pf_cm.__enter__()
        w23 = pfp.tile([P, FFB, C], FP8, tag="w23", name="w23")         # 12KB
        gB3 = pfp.tile([P, FFB, TQ], FP8, tag="gB3", name="gB3")        # 12KB
        nc.sync.dma_start(w23[:, :, :],
                          w2[:, :].rearrange("(f p) n -> p f n", p=P))

        # ---------------- FFN ----------------
        peps_cm = tc.tile_pool(name="pe_ps", bufs=2, space="PSUM")
        peps = peps_cm.__enter__()
        for t2 in range(TQT):
            sq3 = pep.tile([P, CT, NT], FP8, tag="sqe", bufs=1, name="sqe")
            for cp_ in range(CT // 2):
                eng = nc.gpsimd if cp_ != 3 else nc.vector
                eng.tensor_mul(sq3[:, 2 * cp_:2 * cp_ + 2, :],
                               hB[:, 2 * cp_:2 * cp_ + 2, t2 * NT:(t2 + 1) * NT],
                               hB[:, 2 * cp_:2 * cp_ + 2, t2 * NT:(t2 + 1) * NT])
            ss = peps.tile([P, NT], F32, tag="sse", bufs=1, name="sse")
            for cp_ in range(CT // 2):
                nc.tensor.matmul(ss[:], ones8[:], sq3[:, 2 * cp_:2 * cp_ + 2, :],
                                 start=(cp_ == 0), stop=(cp_ == CT // 2 - 1),
                                 perf_mode=DRW)
            sqt = pep.tile([P, NT], F32, tag="sqte", bufs=1, name="sqte")
            nc.scalar.activation(sqt[:], ss[:], AF.Sqrt, scale=1.0 / C, bias=eps_t[:])
            rn = pep.tile([P, NT], F32, tag="rne", bufs=1, name="rne")
            nc.vector.reciprocal(rn[:], sqt[:])
            for ci in range(CT):
                eng = nc.gpsimd if ci % 4 != 3 else nc.vector
                eng.tensor_mul(fB3[:, ci, t2 * NT:(t2 + 1) * NT],
                               hB[:, ci, t2 * NT:(t2 + 1) * NT], rn[:])
        for t2 in range(TQT):
            for fp_ in range(FFB // 2):
                pu = peps.tile([P, 2, NT], F32, tag="pu", bufs=2, name="pu")
                for half in range(2):
                    fb = 2 * fp_ + half
                    for cp_ in range(CT // 2):
                        nc.tensor.matmul(
                            pu[:, half, :],
                            w13[:, 2 * cp_:2 * cp_ + 2, fb * P:(fb + 1) * P],
                            fB3[:, 2 * cp_:2 * cp_ + 2, t2 * NT:(t2 + 1) * NT],
                            start=(cp_ == 0), stop=(cp_ == CT // 2 - 1),
                            perf_mode=DRW)
                nc.scalar.activation(
                    gB3[:, 2 * fp_:2 * fp_ + 2, t2 * NT:(t2 + 1) * NT],
                    pu[:], AF.Gelu, scale=1.0 / WS)
        for t2 in range(TQT):
            yB = pfp.tile([P, CT, NT], F32, tag="yB", bufs=2, name="yB")
            for cb in range(CT):
                py = peps.tile([P, NT], F32, tag="py", bufs=3, name="py")
                for fp_ in range(FFB // 2):
                    nc.tensor.matmul(
                        py[:],
                        w23[:, 2 * fp_:2 * fp_ + 2, cb * P:(cb + 1) * P],
                        gB3[:, 2 * fp_:2 * fp_ + 2, t2 * NT:(t2 + 1) * NT],
                        start=(fp_ == 0), stop=(fp_ == FFB // 2 - 1),
                        perf_mode=DRW)
                nc.vector.scalar_tensor_tensor(
                    yB[:, cb, :], py[:], 1.0 / WS,
                    hB[:, cb, t2 * NT:(t2 + 1) * NT], ALU.mult, ALU.add)
            out_r = out[:, :].rearrange("(c p) t -> p c t", p=P)
            for cq_ in range(CT):
                nc.sync.dma_start(
                    out_r[:, cq_:cq_ + 1, t2 * NT:(t2 + 1) * NT],
                    yB[:, cq_:cq_ + 1, :])
        peps_cm.__exit__(None, None, None)
        pf_cm.__exit__(None, None, None)
        pe_cm.__exit__(None, None, None)
        hR_cm.__exit__(None, None, None)
        xb_cm.__exit__(None, None, None)
        qo_cm.__exit__(None, None, None)
        dram_cm.__exit__(None, None, None)
        cpool_cm.__exit__(None, None, None)

        sched_state, snap = tc.schedule_and_allocate()
        _CACHE["predicted_ns"] = snap.time if snap is not None else None
        try:
            _CACHE["dispatch_ns"] = sched_state.get_inst_dispatch_ns()
        except Exception:
            _CACHE["dispatch_ns"] = None

    nc.finalize()
    return nc


def get_nc():
    if "nc" not in _CACHE:
        _CACHE["nc"] = _build()
    return _CACHE["nc"]


def _prep_inputs(inputs):
    f8 = ml_dtypes.float8_e4m3
    x = np.asarray(inputs["x"], dtype=np.float32)
    g_attn = np.asarray(inputs["g_attn"], np.float32)
    g_ff = np.asarray(inputs["g_ff"], np.float32)
    wq8 = (g_attn[:, None] * np.asarray(inputs["Wq"], np.float32) * WS_QKV).astype(f8)
    wk8 = (g_attn[:, None] * np.asarray(inputs["Wk"], np.float32) * WS_QKV).astype(f8)
    wv8 = (g_attn[:, None] * np.asarray(inputs["Wv"], np.float32) * WS_QKV).astype(f8)
    wo8 = (np.asarray(inputs["Wo"], np.float32) * WS).astype(f8)
    w18 = (g_ff[:, None] * np.asarray(inputs["W1"], np.float32) * WS).astype(f8)
    w28 = (np.asarray(inputs["W2"], np.float32) * WS).astype(f8)
    in_maps = []
    for core in range(8):
        b, cq = divmod(core, 4)
        xc = np.ascontiguousarray(x[b][:, cq * TQ:(cq + 1) * TQ])
        in_maps.append({
            "xqb": xc.astype(ml_dtypes.bfloat16),
            "wq": wq8, "wk": wk8, "wv": wv8, "wo": wo8, "w1": w18, "w2": w28,
        })
    return in_maps


def run(inputs, **kwargs):
    nc = get_nc()
    in_maps = _prep_inputs(inputs)
    res = run_bass_kernel_spmd(nc, in_maps, core_ids=list(range(8)), **kwargs)
    out = np.empty((B, C, T), np.float32)
    for core in range(8):
        b, cq = divmod(core, 4)
        out[b][:, cq * TQ:(cq + 1) * TQ] = res.results[core]["out"]
    return out, res


def kernel(**inputs) -> np.ndarray:
    out, _ = run(inputs)
    return out


# revision 56
# speedup vs baseline: 1.0070x; 1.0054x over previous
"""Trainium2 Bass kernel for a pre-RMSNorm attention+FFN transformer block.

Problem: x (2, 1024, 4096) fp32, channel-major (B, C, T).
  h = x^T; h += Attn(RMSNorm(h)); h += FFN(RMSNorm(h)); return h^T.

Sharding: 8 cores = 2 batches x 4 query-token chunks of 1024.  Each core
computes K/V for its batch's own 1024-token chunk, AllGathers K then V
across the 4-core replica group (one collective each; the cost model
charges a flat ~15us per collective and serializes them, so fewer,
earlier collectives win), then runs attention + Wo + FFN for its chunk.

All big matmuls run in fp8e4 with DoubleRow perf mode.  Weights are
prescaled on the host (x32 for Wq/Wk/Wv, x64 for Wo/W1/W2) so fp8's
normal range is used; scales fold back via activation scale factors and
fused scalar_tensor_tensor residual adds.  The softmax exp is split
between the Act engine (exact exp) and a DVE+Pool Schraudolph bit-trick
pipe so all three elementwise engines run hot; the denominator comes
from a DoubleRow matmul against a constant tile.  The residual path is
bf16 x + f32 accumulation.  DMAs are merged into few large strided
transfers (shared HWDGE serializes per-DMA overhead).
"""

import numpy as np
import ml_dtypes

import concourse.bass as bass
import concourse.mybir as mybir
import concourse.tile as tile
from concourse import bacc
from concourse.bass_utils import run_bass_kernel_spmd

F32 = mybir.dt.float32
BF16 = mybir.dt.bfloat16
FP8 = mybir.dt.float8e4
I32 = mybir.dt.int32
AF = mybir.ActivationFunctionType
ALU = mybir.AluOpType
DRW = mybir.MatmulPerfMode.DoubleRow

B = 2
C = 1024
T = 4096
TQ = 1024          # query-token chunk per core
H = 4
DH = 256
FF = 1536
P = 128
NT = 512           # moving-operand / PSUM tile width
CT = C // P        # 8 channel tiles
TQT = TQ // NT     # 2 chunk token tiles
DB = C // P        # 8 output-channel blocks for q/k/v/o
FFB = FF // P      # 12 ff blocks
TJ = T // P        # 32 key-token blocks
JP = TJ // 2       # 16 key-block pairs

WS_QKV = 32.0      # host prescale on Wq/Wk/Wv
WS = 64.0          # host prescale on Wo/W1/W2
OSC = 16.0         # scale of oT relative to true attention output
ONES_DEN = WS_QKV / OSC              # memset value for the denominator matmul
EXP_SCALE = (DH ** -0.5) / (WS_QKV * WS_QKV)
SCH_A = 12102203.161561485           # 2^23 / ln 2
SCH_B = 127.0 * (1 << 23) - 366000.0
SCH_SET = {2, 5, 8, 10, 12, 14}  # pairs routed to the DVE+Pool exp pipe

_CACHE = {}


def _build():
    nc = bacc.Bacc()
    xqb = nc.dram_tensor("xqb", [C, TQ], BF16, kind="ExternalInput")
    wq = nc.dram_tensor("wq", [C, C], FP8, kind="ExternalInput")
    wk = nc.dram_tensor("wk", [C, C], FP8, kind="ExternalInput")
    wv = nc.dram_tensor("wv", [C, C], FP8, kind="ExternalInput")
    wo = nc.dram_tensor("wo", [C, C], FP8, kind="ExternalInput")
    w1 = nc.dram_tensor("w1", [C, FF], FP8, kind="ExternalInput")
    w2 = nc.dram_tensor("w2", [FF, C], FP8, kind="ExternalInput")
    out = nc.dram_tensor("out", [C, TQ], F32, kind="ExternalOutput")

    RG = [[0, 1, 2, 3], [4, 5, 6, 7]]

    with tile.TileContext(nc) as tc:
        cpool_cm = tc.tile_pool(name="const", bufs=1)
        cpool = cpool_cm.__enter__()
        ones8 = cpool.tile([P, 2, P], FP8, tag="ones8", name="ones8")
        nc.vector.memset(ones8[:], 1.0)
        ones_b = cpool.tile([P, P], BF16, tag="ones_b", name="ones_b")
        nc.vector.memset(ones_b[:], 1.0)
        ones_d = cpool.tile([P, 2, P], FP8, tag="ones_d", name="ones_d")
        nc.vector.memset(ones_d[:], ONES_DEN)
        eps_t = cpool.tile([P, 1], F32, tag="eps", name="eps_t")
        nc.vector.memset(eps_t[:], 1e-8)

        dram_cm = tc.tile_pool(name="dram", bufs=1, space="DRAM")
        dp = dram_cm.__enter__()
        kl_d = dp.tile([C, TQ], FP8, tag="kl_d", name="kl_d")
        vl_d = dp.tile([TQ, C], FP8, tag="vl_d", name="vl_d")
        kg_d = dp.tile([4 * C, TQ], FP8, tag="kg_d", name="kg_d")
        vg_d = dp.tile([4 * TQ, C], FP8, tag="vg_d", name="vg_d")

        # ---- persistent SBUF (right side) ----
        qo_cm = tc.tile_pool(name="qopool", bufs=1, side="right")
        qop = qo_cm.__enter__()
        qT3 = qop.tile([P, DB, TQ], FP8, tag="qT", name="qT3")          # 8KB
        oT3 = qT3  # o^T reuses q^T: each (head, ti) slice is dead after scores

        xb_cm = tc.tile_pool(name="xbpool", bufs=1, side="right")
        xbp = xb_cm.__enter__()
        xbT = xbp.tile([P, CT, TQ], BF16, tag="xbT", name="xbT")        # 16KB

        pbA_cm = tc.tile_pool(name="pbA", bufs=1, side="right")
        pbA = pbA_cm.__enter__()
        wq3 = pbA.tile([P, CT, C], FP8, tag="wq3", name="wq3")
        wk3 = pbA.tile([P, CT, C], FP8, tag="wk3", name="wk3")
        wv3 = pbA.tile([P, CT, C], FP8, tag="wv3", name="wv3")
        aT3 = pbA.tile([P, CT, TQ], FP8, tag="aT3", name="aT3")

        # ---- input + weight DMAs (merged, ordered by need) ----
        xqb_r = xqb[:, :].rearrange("(c p) t -> p c t", p=P)
        nc.sync.dma_start(xbT[:, 0:CT // 2, 0:NT], xqb_r[:, 0:CT // 2, 0:NT])
        nc.sync.dma_start(xbT[:, CT // 2:CT, 0:NT], xqb_r[:, CT // 2:CT, 0:NT])
        nc.sync.dma_start(xbT[:, 0:CT // 2, NT:TQ], xqb_r[:, 0:CT // 2, NT:TQ])
        nc.sync.dma_start(xbT[:, CT // 2:CT, NT:TQ], xqb_r[:, CT // 2:CT, NT:TQ])
        nc.sync.dma_start(wk3[:, :, :],
                          wk[:, :].rearrange("(c p) n -> p c n", p=P))
        nc.sync.dma_start(wv3[:, :, :],
                          wv[:, :].rearrange("(c p) n -> p c n", p=P))

        pbN_cm = tc.tile_pool(name="pbN", bufs=1)
        pbN = pbN_cm.__enter__()
        pbps_cm = tc.tile_pool(name="pb_ps", bufs=1, space="PSUM")
        pbps = pbps_cm.__enter__()

        # ---- chunk rmsnorm -> aT3 (fp8) ----
        for t2 in range(TQT):
            x_sl = xbT[:, :, t2 * NT:(t2 + 1) * NT]
            sq3 = pbN.tile([P, CT, NT], BF16, tag="sqb", bufs=2, name="sqb")
            for cp_ in range(CT // 2):
                eng = nc.vector if cp_ % 2 == 0 else nc.gpsimd
                eng.tensor_mul(sq3[:, 2 * cp_:2 * cp_ + 2, :],
                               x_sl[:, 2 * cp_:2 * cp_ + 2, :],
                               x_sl[:, 2 * cp_:2 * cp_ + 2, :])
            ss = pbps.tile([P, NT], F32, tag="ssb", bufs=2, name="ssb")
            for ci in range(CT):
                nc.tensor.matmul(ss[:], ones_b[:], sq3[:, ci, :],
                                 start=(ci == 0), stop=(ci == CT - 1))
            sqt = pbN.tile([P, NT], F32, tag="sqtb", bufs=2, name="sqtb")
            nc.scalar.activation(sqt[:], ss[:], AF.Sqrt, scale=1.0 / C, bias=eps_t[:])
            rn = pbN.tile([P, NT], F32, tag="rnb", bufs=2, name="rnb")
            nc.vector.reciprocal(rn[:], sqt[:])
            for ci in range(CT):
                eng = nc.gpsimd if ci % 8 < 5 else nc.vector
                eng.tensor_mul(aT3[:, ci, t2 * NT:(t2 + 1) * NT],
                               x_sl[:, ci, :], rn[:])

        # ---- K/V staging in SBUF, 2 store DMAs, 2 collectives ----
        kvs_cm = tc.tile_pool(name="kvs", bufs=1)
        kvs = kvs_cm.__enter__()
        k8 = kvs.tile([P, DB, TQ], FP8, tag="k8", name="k8")            # 8KB
        v8 = kvs.tile([P, TQ // P, C], FP8, tag="v8", name="v8")        # 8KB

        cp_engs = [nc.scalar, nc.vector, nc.scalar, nc.vector]

        for dp_ in range(DB // 2):
            for t2 in range(TQT):
                pk = pbps.tile([P, 2, NT], F32, tag="pp", bufs=3, name="pk")
                for half in range(2):
                    db = 2 * dp_ + half
                    for cp_ in range(CT // 2):
                        nc.tensor.matmul(
                            pk[:, half, :],
                            wk3[:, 2 * cp_:2 * cp_ + 2, db * P:(db + 1) * P],
                            aT3[:, 2 * cp_:2 * cp_ + 2, t2 * NT:(t2 + 1) * NT],
                            start=(cp_ == 0), stop=(cp_ == CT // 2 - 1),
                            perf_mode=DRW)
                nc.scalar.copy(
                    k8[:, 2 * dp_, t2 * NT:(t2 + 1) * NT], pk[:, 0, :])
                nc.vector.tensor_copy(
                    k8[:, 2 * dp_ + 1, t2 * NT:(t2 + 1) * NT], pk[:, 1, :])
                nc.sync.dma_start(
                    kl_d[:, :].rearrange("(c p) t -> p c t", p=P)
                    [:, 2 * dp_:2 * dp_ + 2, t2 * NT:(t2 + 1) * NT],
                    k8[:, 2 * dp_:2 * dp_ + 2, t2 * NT:(t2 + 1) * NT])

        for jl in range(TQ // P):
            pv = pbps.tile([P, 2, NT], F32, tag="pp", bufs=3, name="pv")
            for hf in range(2):
                for cp_ in range(CT // 2):
                    nc.tensor.matmul(
                        pv[:, hf, :],
                        aT3[:, 2 * cp_:2 * cp_ + 2, jl * P:(jl + 1) * P],
                        wv3[:, 2 * cp_:2 * cp_ + 2, hf * NT:(hf + 1) * NT],
                        start=(cp_ == 0), stop=(cp_ == CT // 2 - 1),
                        perf_mode=DRW)
            eng = cp_engs[jl % 4]
            if eng is nc.scalar:
                nc.scalar.copy(v8[:, jl, :], pv[:])
            else:
                eng.tensor_copy(v8[:, jl, :], pv[:])
            if jl % 2 == 1:
                nc.sync.dma_start(
                    vl_d[:, :].rearrange("(j p) c -> p j c", p=P)[:, jl - 1:jl + 1, :],
                    v8[:, jl - 1:jl + 1, :])

        nc.gpsimd.collective_compute(
            "AllGather", mybir.AluOpType.bypass, replica_groups=RG,
            ins=[kl_d[:, :]], outs=[kg_d[:, :]])
        nc.gpsimd.collective_compute(
            "AllGather", mybir.AluOpType.bypass, replica_groups=RG,
            ins=[vl_d[:, :]], outs=[vg_d[:, :]])

        # ---- Q (overlaps the collectives) ----
        nc.sync.dma_start(wq3[:, :, :],
                          wq[:, :].rearrange("(c p) n -> p c n", p=P))
        for t2 in range(TQT):
            for dp_ in range(DB // 2):
                pq = pbps.tile([P, 2, NT], F32, tag="pp", bufs=3, name="pq")
                for half in range(2):
                    db = 2 * dp_ + half
                    for cp_ in range(CT // 2):
                        nc.tensor.matmul(
                            pq[:, half, :],
                            wq3[:, 2 * cp_:2 * cp_ + 2, db * P:(db + 1) * P],
                            aT3[:, 2 * cp_:2 * cp_ + 2, t2 * NT:(t2 + 1) * NT],
                            start=(cp_ == 0), stop=(cp_ == CT // 2 - 1),
                            perf_mode=DRW)
                nc.scalar.copy(
                    qT3[:, 2 * dp_:2 * dp_ + 2, t2 * NT:(t2 + 1) * NT], pq[:])

        kvs_cm.__exit__(None, None, None)
        pbN_cm.__exit__(None, None, None)
        pbps_cm.__exit__(None, None, None)
        pbA_cm.__exit__(None, None, None)

        # ---- more weights during the collective window ----
        hR_cm = tc.tile_pool(name="hpool", bufs=1, side="right")
        hRp = hR_cm.__enter__()
        hB = hRp.tile([P, CT, TQ], F32, tag="hB", name="hB")            # 32KB
        pe_cm = tc.tile_pool(name="pe", bufs=1, side="right")
        pep = pe_cm.__enter__()
        w13 = pep.tile([P, CT, FF], FP8, tag="w13", name="w13")         # 12KB
        nc.scalar.dma_start(w13[:, :, :],
                            w1[:, :].rearrange("(c p) n -> p c n", p=P))
        fB3 = pep.tile([P, CT, TQ], FP8, tag="fB3", name="fB3")         # 8KB
        wo_cm = tc.tile_pool(name="wopool", bufs=1, side="right")
        wop = wo_cm.__enter__()
        wo3 = wop.tile([P, CT, C], FP8, tag="wo3", name="wo3")
        nc.scalar.dma_start(wo3[:, :, :],
                            wo[:, :].rearrange("(c p) n -> p c n", p=P))

        # ---- gathered K/V reload: K first (scores need it), then V ----
        kT_cm = tc.tile_pool(name="kTpool", bufs=1)
        kTp = kT_cm.__enter__()
        kT3 = kTp.tile([P, DB, T], FP8, tag="kT", name="kT3")           # 32KB
        vB_cm = tc.tile_pool(name="vBpool", bufs=1)
        vBp = vB_cm.__enter__()
        vB3 = vBp.tile([P, TJ, C], FP8, tag="vB", name="vB3")           # 32KB

        kg_r = kg_d[:, :].rearrange("(r c p) t -> p r c t", p=P, r=4)
        for hp in range(H):
            for r in range(4):
                nc.sync.dma_start(
                    kT3[:, 2 * hp:2 * hp + 2, r * TQ:(r + 1) * TQ],
                    kg_r[:, r, 2 * hp:2 * hp + 2, :])
        vg_r = vg_d[:, :].rearrange("(r j p) c -> p r j c", p=P, r=4)
        for r in range(4):
            nc.sync.dma_start(vB3[:, r * (TQ // P):(r + 1) * (TQ // P), 0:NT],
                              vg_r[:, r, :, 0:NT])
        for r in range(4):
            nc.sync.dma_start(vB3[:, r * (TQ // P):(r + 1) * (TQ // P), NT:C],
                              vg_r[:, r, :, NT:C])

        # ---------------- attention (+ interleaved Wo/residual) ----------------
        pc_cm = tc.tile_pool(name="pc", bufs=1)
        pcp = pc_cm.__enter__()
        pss_cm = tc.tile_pool(name="ps_s", bufs=2, space="PSUM")
        pss = pss_cm.__enter__()
        pso_cm = tc.tile_pool(name="ps_o", bufs=1, space="PSUM")
        pso = pso_cm.__enter__()
        for ti in range(TQT):
            for h in range(H):
                et3 = pcp.tile([P, TJ, NT], FP8, tag="exp", bufs=2, name="et3")
                q_sl = qT3[:, 2 * h:2 * h + 2, ti * NT:(ti + 1) * NT]
                HN = NT // 2
                for jp in range(JP):
                    for qh in range(2):
                        psc = pss.tile([P, 2, HN], F32, tag="s", bufs=4,
                                       name="psc")
                        for half in range(2):
                            tj = 2 * jp + half
                            nc.tensor.matmul(
                                psc[:, half, :],
                                kT3[:, 2 * h:2 * h + 2, tj * P:(tj + 1) * P],
                                q_sl[:, :, qh * HN:(qh + 1) * HN],
                                start=True, stop=True, perf_mode=DRW)
                        e_sl = et3[:, 2 * jp:2 * jp + 2, qh * HN:(qh + 1) * HN]
                        if jp in SCH_SET or (jp == 6 and qh == 1):
                            sch = pcp.tile([P, 2, HN], I32, tag="sch", bufs=8,
                                           name="sch")
                            nc.vector.tensor_scalar(sch[:], psc[:],
                                                    SCH_A * EXP_SCALE, SCH_B,
                                                    ALU.mult, ALU.add)
                            nc.gpsimd.tensor_copy(e_sl, sch[:].bitcast(F32))
                        else:
                            nc.scalar.activation(e_sl, psc[:], AF.Exp,
                                                 scale=EXP_SCALE)
                po0 = pso.tile([P, NT], F32, tag="po0", name="po0")
                po1 = pso.tile([P, NT], F32, tag="po1", name="po1")
                pr = pso.tile([P, NT], F32, tag="pr", name="pr")
                for jp in range(JP):
                    e_sl = et3[:, 2 * jp:2 * jp + 2, :]
                    st_, sp_ = (jp == 0), (jp == JP - 1)
                    nc.tensor.matmul(pr[:], ones_d[:], e_sl, start=st_, stop=sp_,
                                     perf_mode=DRW, skip_group_check=True)
                    nc.tensor.matmul(po0[:],
                                     vB3[:, 2 * jp:2 * jp + 2, h * DH: h * DH + P],
                                     e_sl, start=st_, stop=sp_,
                                     perf_mode=DRW, skip_group_check=True)
                    nc.tensor.matmul(po1[:],
                                     vB3[:, 2 * jp:2 * jp + 2, h * DH + P:(h + 1) * DH],
                                     e_sl, start=st_, stop=sp_,
                                     perf_mode=DRW, skip_group_check=True)
                rec = pcp.tile([P, NT], F32, tag="rec", bufs=1, name="rec")
                nc.vector.reciprocal(rec[:], pr[:])
                nc.vector.tensor_mul(oT3[:, 2 * h, ti * NT:(ti + 1) * NT],
                                     po0[:], rec[:])
                nc.vector.tensor_mul(oT3[:, 2 * h + 1, ti * NT:(ti + 1) * NT],
                                     po1[:], rec[:])
            # ---- Wo + residual for this token half ----
            t2 = ti
            for cb in range(CT):
                if t2 == TQT - 1:
                    ph_tag = ("ph", "po0", "po1")[cb % 3]
                else:
                    ph_tag = "ph"
                ph = pso.tile([P, NT], F32, tag=ph_tag, bufs=1, name="ph")
                for cp_ in range(CT // 2):
                    nc.tensor.matmul(
                        ph[:],
                        wo3[:, 2 * cp_:2 * cp_ + 2, cb * P:(cb + 1) * P],
                        oT3[:, 2 * cp_:2 * cp_ + 2, t2 * NT:(t2 + 1) * NT],
                        start=(cp_ == 0), stop=(cp_ == CT // 2 - 1),
                        perf_mode=DRW)
                nc.vector.scalar_tensor_tensor(
                    hB[:, cb, t2 * NT:(t2 + 1) * NT], ph[:],
                    1.0 / (OSC * WS), xbT[:, cb, t2 * NT:(t2 + 1) * NT],
                    ALU.mult, ALU.add)
        pso_cm.__exit__(None, None, None)
        pss_cm.__exit__(None, None, None)
        pc_cm.__exit__(None, None, None)
        vB_cm.__exit__(None, None, None)
        kT_cm.__exit__(None, None, None)
        wo_cm.__exit__(None, None, None)

        # w23 loads into the space freed by the attention pools
        pf_cm = tc.tile_pool(name="pf", bufs=1)
        pfp = pf_cm.__enter__()
        w23 = pfp.tile([P, FFB, C], FP8, tag="w23", name="w23")         # 12KB
        gB3 = pfp.tile([P, FFB, TQ], FP8, tag="gB3", name="gB3")        # 12KB
        nc.sync.dma_start(w23[:, :, :],
                          w2[:, :].rearrange("(f p) n -> p f n", p=P))

        # ---------------- FFN ----------------
        peps_cm = tc.tile_pool(name="pe_ps", bufs=2, space="PSUM")
        peps = peps_cm.__enter__()
        for t2 in range(TQT):
            sq3 = pep.tile([P, CT, NT], FP8, tag="sqe", bufs=1, name="sqe")
            for cp_ in range(CT // 2):
                eng = nc.gpsimd if cp_ != 3 else nc.vector
                eng.tensor_mul(sq3[:, 2 * cp_:2 * cp_ + 2, :],
                               hB[:, 2 * cp_:2 * cp_ + 2, t2 * NT:(t2 + 1) * NT],
                               hB[:, 2 * cp_:2 * cp_ + 2, t2 * NT:(t2 + 1) * NT])
            ss = peps.tile([P, NT], F32, tag="sse", bufs=1, name="sse")
            for cp_ in range(CT // 2):
                nc.tensor.matmul(ss[:], ones8[:], sq3[:, 2 * cp_:2 * cp_ + 2, :],
                                 start=(cp_ == 0), stop=(cp_ == CT // 2 - 1),
                                 perf_mode=DRW)
            sqt = pep.tile([P, NT], F32, tag="sqte", bufs=1, name="sqte")
            nc.scalar.activation(sqt[:], ss[:], AF.Sqrt, scale=1.0 / C, bias=eps_t[:])
            rn = pep.tile([P, NT], F32, tag="rne", bufs=1, name="rne")
            nc.vector.reciprocal(rn[:], sqt[:])
            for ci in range(CT):
                eng = nc.gpsimd if ci % 4 != 3 else nc.vector
                eng.tensor_mul(fB3[:, ci, t2 * NT:(t2 + 1) * NT],
                               hB[:, ci, t2 * NT:(t2 + 1) * NT], rn[:])
        for t2 in range(TQT):
            for fp_ in range(FFB // 2):
                pu = peps.tile([P, 2, NT], F32, tag="pu", bufs=2, name="pu")
                for half in range(2):
                    fb = 2 * fp_ + half
                    for cp_ in range(CT // 2):
                        nc.tensor.matmul(
                            pu[:, half, :],
                            w13[:, 2 * cp_:2 * cp_ + 2, fb * P:(fb + 1) * P],
                            fB3[:, 2 * cp_:2 * cp_ + 2, t2 * NT:(t2 + 1) * NT],
                            start=(cp_ == 0), stop=(cp_ == CT // 2 - 1),
                            perf_mode=DRW)
                nc.scalar.activation(
                    gB3[:, 2 * fp_:2 * fp_ + 2, t2 * NT:(t2 + 1) * NT],
                    pu[:], AF.Gelu, scale=1.0 / WS)
        for t2 in range(TQT):
            yB = pfp.tile([P, CT, NT], F32, tag="yB", bufs=2, name="yB")
            for cb in range(CT):
                py = peps.tile([P, NT], F32, tag="py", bufs=3, name="py")
                for fp_ in range(FFB // 2):
                    nc.tensor.matmul(
                        py[:],
                        w23[:, 2 * fp_:2 * fp_ + 2, cb * P:(cb + 1) * P],
                        gB3[:, 2 * fp_:2 * fp_ + 2, t2 * NT:(t2 + 1) * NT],
                        start=(fp_ == 0), stop=(fp_ == FFB // 2 - 1),
                        perf_mode=DRW)
                nc.vector.scalar_tensor_tensor(
                    yB[:, cb, :], py[:], 1.0 / WS,
                    hB[:, cb, t2 * NT:(t2 + 1) * NT], ALU.mult, ALU.add)
            out_r = out[:, :].rearrange("(c p) t -> p c t", p=P)
            for cq_ in range(CT):
                nc.sync.dma_start(
                    out_r[:, cq_:cq_ + 1, t2 * NT:(t2 + 1) * NT],
                    yB[:, cq_:cq_ + 1, :])
        peps_cm.__exit__(None, None, None)
        pf_cm.__exit__(None, None, None)
        pe_cm.__exit__(None, None, None)
        hR_cm.__exit__(None, None, None)
        xb_cm.__exit__(None, None, None)
        qo_cm.__exit__(None, None, None)
        dram_cm.__exit__(None, None, None)
        cpool_cm.__exit__(None, None, None)

        sched_state, snap = tc.schedule_and_allocate()
        _CACHE["predicted_ns"] = snap.time if snap is not None else None
        try:
            _CACHE["dispatch_ns"] = sched_state.get_inst_dispatch_ns()
        except Exception:
            _CACHE["dispatch_ns"] = None

    nc.finalize()
    return nc


def get_nc():
    if "nc" not in _CACHE:
        _CACHE["nc"] = _build()
    return _CACHE["nc"]


def _prep_inputs(inputs):
    f8 = ml_dtypes.float8_e4m3
    x = np.asarray(inputs["x"], dtype=np.float32)
    g_attn = np.asarray(inputs["g_attn"], np.float32)
    g_ff = np.asarray(inputs["g_ff"], np.float32)
    wq8 = (g_attn[:, None] * np.asarray(inputs["Wq"], np.float32) * WS_QKV).astype(f8)
    wk8 = (g_attn[:, None] * np.asarray(inputs["Wk"], np.float32) * WS_QKV).astype(f8)
    wv8 = (g_attn[:, None] * np.asarray(inputs["Wv"], np.float32) * WS_QKV).astype(f8)
    wo8 = (np.asarray(inputs["Wo"], np.float32) * WS).astype(f8)
    w18 = (g_ff[:, None] * np.asarray(inputs["W1"], np.float32) * WS).astype(f8)
    w28 = (np.asarray(inputs["W2"], np.float32) * WS).astype(f8)
    in_maps = []
    for core in range(8):
        b, cq = divmod(core, 4)
        xc = np.ascontiguousarray(x[b][:, cq * TQ:(cq + 1) * TQ])
        in_maps.append({
            "xqb": xc.astype(ml_dtypes.bfloat16),
            "wq": wq8, "wk": wk8, "wv": wv8, "wo": wo8, "w1": w18, "w2": w28,
        })
    return in_maps


def run(inputs, **kwargs):
    nc = get_nc()
    in_maps = _prep_inputs(inputs)
    res = run_bass_kernel_spmd(nc, in_maps, core_ids=list(range(8)), **kwargs)
    out = np.empty((B, C, T), np.float32)
    for core in range(8):
        b, cq = divmod(core, 4)
        out[b][:, cq * TQ:(cq + 1) * TQ] = res.results[core]["out"]
    return out, res


def kernel(**inputs) -> np.ndarray:
    out, _ = run(inputs)
    return out


# revision 57
# speedup vs baseline: 1.0087x; 1.0017x over previous
"""Trainium2 Bass kernel for a pre-RMSNorm attention+FFN transformer block.

Problem: x (2, 1024, 4096) fp32, channel-major (B, C, T).
  h = x^T; h += Attn(RMSNorm(h)); h += FFN(RMSNorm(h)); return h^T.

Sharding: 8 cores = 2 batches x 4 query-token chunks of 1024.  Each core
computes K/V for its batch's own 1024-token chunk, AllGathers K then V
across the 4-core replica group (one collective each; the cost model
charges a flat ~15us per collective and serializes them, so fewer,
earlier collectives win), then runs attention + Wo + FFN for its chunk.

All big matmuls run in fp8e4 with DoubleRow perf mode.  Weights are
prescaled on the host (x32 for Wq/Wk/Wv, x64 for Wo/W1/W2) so fp8's
normal range is used; scales fold back via activation scale factors and
fused scalar_tensor_tensor residual adds.  The softmax exp is split
between the Act engine (exact exp) and a DVE+Pool Schraudolph bit-trick
pipe so all three elementwise engines run hot; the denominator comes
from a DoubleRow matmul against a constant tile.  The residual path is
bf16 x + f32 accumulation.  DMAs are merged into few large strided
transfers (shared HWDGE serializes per-DMA overhead).
"""

import numpy as np
import ml_dtypes

import concourse.bass as bass
import concourse.mybir as mybir
import concourse.tile as tile
from concourse import bacc
from concourse.bass_utils import run_bass_kernel_spmd

F32 = mybir.dt.float32
BF16 = mybir.dt.bfloat16
FP8 = mybir.dt.float8e4
I32 = mybir.dt.int32
AF = mybir.ActivationFunctionType
ALU = mybir.AluOpType
DRW = mybir.MatmulPerfMode.DoubleRow

B = 2
C = 1024
T = 4096
TQ = 1024          # query-token chunk per core
H = 4
DH = 256
FF = 1536
P = 128
NT = 512           # moving-operand / PSUM tile width
CT = C // P        # 8 channel tiles
TQT = TQ // NT     # 2 chunk token tiles
DB = C // P        # 8 output-channel blocks for q/k/v/o
FFB = FF // P      # 12 ff blocks
TJ = T // P        # 32 key-token blocks
JP = TJ // 2       # 16 key-block pairs

WS_QKV = 32.0      # host prescale on Wq/Wk/Wv
WS = 64.0          # host prescale on Wo/W1/W2
OSC = 16.0         # scale of oT relative to true attention output
ONES_DEN = WS_QKV / OSC              # memset value for the denominator matmul
EXP_SCALE = (DH ** -0.5) / (WS_QKV * WS_QKV)
SCH_A = 12102203.161561485           # 2^23 / ln 2
SCH_B = 127.0 * (1 << 23) - 366000.0
SCH_SET = {2, 5, 8, 10, 12, 14}  # pairs routed to the DVE+Pool exp pipe

_CACHE = {}


def _build():
    nc = bacc.Bacc()
    xqb = nc.dram_tensor("xqb", [C, TQ], BF16, kind="ExternalInput")
    wq = nc.dram_tensor("wq", [C, C], FP8, kind="ExternalInput")
    wk = nc.dram_tensor("wk", [C, C], FP8, kind="ExternalInput")
    wv = nc.dram_tensor("wv", [C, C], FP8, kind="ExternalInput")
    wo = nc.dram_tensor("wo", [C, C], FP8, kind="ExternalInput")
    w1 = nc.dram_tensor("w1", [C, FF], FP8, kind="ExternalInput")
    w2 = nc.dram_tensor("w2", [FF, C], FP8, kind="ExternalInput")
    out = nc.dram_tensor("out", [C, TQ], F32, kind="ExternalOutput")

    RG = [[0, 1, 2, 3], [4, 5, 6, 7]]

    with tile.TileContext(nc) as tc:
        cpool_cm = tc.tile_pool(name="const", bufs=1)
        cpool = cpool_cm.__enter__()
        ones8 = cpool.tile([P, 2, P], FP8, tag="ones8", name="ones8")
        nc.vector.memset(ones8[:], 1.0)
        ones_b = cpool.tile([P, P], BF16, tag="ones_b", name="ones_b")
        nc.vector.memset(ones_b[:], 1.0)
        ones_d = cpool.tile([P, 2, P], FP8, tag="ones_d", name="ones_d")
        nc.vector.memset(ones_d[:], ONES_DEN)
        eps_t = cpool.tile([P, 1], F32, tag="eps", name="eps_t")
        nc.vector.memset(eps_t[:], 1e-8)

        dram_cm = tc.tile_pool(name="dram", bufs=1, space="DRAM")
        dp = dram_cm.__enter__()
        kl_d = dp.tile([C, TQ], FP8, tag="kl_d", name="kl_d")
        vl_d = dp.tile([TQ, C], FP8, tag="vl_d", name="vl_d")
        kg_d = dp.tile([4 * C, TQ], FP8, tag="kg_d", name="kg_d")
        vg_d = dp.tile([4 * TQ, C], FP8, tag="vg_d", name="vg_d")

        # ---- persistent SBUF (right side) ----
        qo_cm = tc.tile_pool(name="qopool", bufs=1, side="right")
        qop = qo_cm.__enter__()
        qT3 = qop.tile([P, DB, TQ], FP8, tag="qT", name="qT3")          # 8KB
        oT3 = qT3  # o^T reuses q^T: each (head, ti) slice is dead after scores

        xb_cm = tc.tile_pool(name="xbpool", bufs=1, side="right")
        xbp = xb_cm.__enter__()
        xbT = xbp.tile([P, CT, TQ], BF16, tag="xbT", name="xbT")        # 16KB

        pbA_cm = tc.tile_pool(name="pbA", bufs=1, side="right")
        pbA = pbA_cm.__enter__()
        wq3 = pbA.tile([P, CT, C], FP8, tag="wq3", name="wq3")
        wk3 = pbA.tile([P, CT, C], FP8, tag="wk3", name="wk3")
        wv3 = pbA.tile([P, CT, C], FP8, tag="wv3", name="wv3")
        aT3 = pbA.tile([P, CT, TQ], FP8, tag="aT3", name="aT3")

        # ---- input + weight DMAs (merged, ordered by need) ----
        xqb_r = xqb[:, :].rearrange("(c p) t -> p c t", p=P)
        nc.sync.dma_start(xbT[:, 0:CT // 2, 0:NT], xqb_r[:, 0:CT // 2, 0:NT])
        nc.sync.dma_start(xbT[:, CT // 2:CT, 0:NT], xqb_r[:, CT // 2:CT, 0:NT])
        nc.sync.dma_start(xbT[:, 0:CT // 2, NT:TQ], xqb_r[:, 0:CT // 2, NT:TQ])
        nc.sync.dma_start(xbT[:, CT // 2:CT, NT:TQ], xqb_r[:, CT // 2:CT, NT:TQ])
        nc.sync.dma_start(wk3[:, :, :],
                          wk[:, :].rearrange("(c p) n -> p c n", p=P))
        nc.sync.dma_start(wv3[:, :, :],
                          wv[:, :].rearrange("(c p) n -> p c n", p=P))

        pbN_cm = tc.tile_pool(name="pbN", bufs=1)
        pbN = pbN_cm.__enter__()
        pbps_cm = tc.tile_pool(name="pb_ps", bufs=1, space="PSUM")
        pbps = pbps_cm.__enter__()

        # ---- chunk rmsnorm -> aT3 (fp8) ----
        for t2 in range(TQT):
            x_sl = xbT[:, :, t2 * NT:(t2 + 1) * NT]
            sq3 = pbN.tile([P, CT, NT], BF16, tag="sqb", bufs=2, name="sqb")
            for cp_ in range(CT // 2):
                eng = nc.vector if cp_ % 2 == 0 else nc.gpsimd
                eng.tensor_mul(sq3[:, 2 * cp_:2 * cp_ + 2, :],
                               x_sl[:, 2 * cp_:2 * cp_ + 2, :],
                               x_sl[:, 2 * cp_:2 * cp_ + 2, :])
            ss = pbps.tile([P, NT], F32, tag="ssb", bufs=2, name="ssb")
            for ci in range(CT):
                nc.tensor.matmul(ss[:], ones_b[:], sq3[:, ci, :],
                                 start=(ci == 0), stop=(ci == CT - 1))
            sqt = pbN.tile([P, NT], F32, tag="sqtb", bufs=2, name="sqtb")
            nc.scalar.activation(sqt[:], ss[:], AF.Sqrt, scale=1.0 / C, bias=eps_t[:])
            rn = pbN.tile([P, NT], F32, tag="rnb", bufs=2, name="rnb")
            nc.vector.reciprocal(rn[:], sqt[:])
            for ci in range(CT):
                eng = nc.gpsimd if ci % 8 < 5 else nc.vector
                eng.tensor_mul(aT3[:, ci, t2 * NT:(t2 + 1) * NT],
                               x_sl[:, ci, :], rn[:])

        # ---- K/V staging in SBUF, 2 store DMAs, 2 collectives ----
        kvs_cm = tc.tile_pool(name="kvs", bufs=1)
        kvs = kvs_cm.__enter__()
        k8 = kvs.tile([P, DB, TQ], FP8, tag="k8", name="k8")            # 8KB
        v8 = kvs.tile([P, TQ // P, C], FP8, tag="v8", name="v8")        # 8KB

        cp_engs = [nc.scalar, nc.vector, nc.scalar, nc.vector]

        for dp_ in range(DB // 2):
            for t2 in range(TQT):
                pk = pbps.tile([P, 2, NT], F32, tag="pp", bufs=3, name="pk")
                for half in range(2):
                    db = 2 * dp_ + half
                    for cp_ in range(CT // 2):
                        nc.tensor.matmul(
                            pk[:, half, :],
                            wk3[:, 2 * cp_:2 * cp_ + 2, db * P:(db + 1) * P],
                            aT3[:, 2 * cp_:2 * cp_ + 2, t2 * NT:(t2 + 1) * NT],
                            start=(cp_ == 0), stop=(cp_ == CT // 2 - 1),
                            perf_mode=DRW)
                nc.scalar.copy(
                    k8[:, 2 * dp_, t2 * NT:(t2 + 1) * NT], pk[:, 0, :])
                nc.vector.tensor_copy(
                    k8[:, 2 * dp_ + 1, t2 * NT:(t2 + 1) * NT], pk[:, 1, :])
                nc.sync.dma_start(
                    kl_d[:, :].rearrange("(c p) t -> p c t", p=P)
                    [:, 2 * dp_:2 * dp_ + 2, t2 * NT:(t2 + 1) * NT],
                    k8[:, 2 * dp_:2 * dp_ + 2, t2 * NT:(t2 + 1) * NT])

        for jl in range(TQ // P):
            pv = pbps.tile([P, 2, NT], F32, tag="pp", bufs=3, name="pv")
            for hf in range(2):
                for cp_ in range(CT // 2):
                    nc.tensor.matmul(
                        pv[:, hf, :],
                        aT3[:, 2 * cp_:2 * cp_ + 2, jl * P:(jl + 1) * P],
                        wv3[:, 2 * cp_:2 * cp_ + 2, hf * NT:(hf + 1) * NT],
                        start=(cp_ == 0), stop=(cp_ == CT // 2 - 1),
                        perf_mode=DRW)
            eng = cp_engs[jl % 4]
            if eng is nc.scalar:
                nc.scalar.copy(v8[:, jl, :], pv[:])
            else:
                eng.tensor_copy(v8[:, jl, :], pv[:])
            if jl % 2 == 1:
                nc.sync.dma_start(
                    vl_d[:, :].rearrange("(j p) c -> p j c", p=P)[:, jl - 1:jl + 1, :],
                    v8[:, jl - 1:jl + 1, :])

        nc.gpsimd.collective_compute(
            "AllGather", mybir.AluOpType.bypass, replica_groups=RG,
            ins=[kl_d[:, :]], outs=[kg_d[:, :]])
        nc.gpsimd.collective_compute(
            "AllGather", mybir.AluOpType.bypass, replica_groups=RG,
            ins=[vl_d[:, :]], outs=[vg_d[:, :]])

        # ---- Q (overlaps the collectives) ----
        nc.sync.dma_start(wq3[:, :, :],
                          wq[:, :].rearrange("(c p) n -> p c n", p=P))
        for t2 in range(TQT):
            for dp_ in range(DB // 2):
                pq = pbps.tile([P, 2, NT], F32, tag="pp", bufs=3, name="pq")
                for half in range(2):
                    db = 2 * dp_ + half
                    for cp_ in range(CT // 2):
                        nc.tensor.matmul(
                            pq[:, half, :],
                            wq3[:, 2 * cp_:2 * cp_ + 2, db * P:(db + 1) * P],
                            aT3[:, 2 * cp_:2 * cp_ + 2, t2 * NT:(t2 + 1) * NT],
                            start=(cp_ == 0), stop=(cp_ == CT // 2 - 1),
                            perf_mode=DRW)
                nc.scalar.copy(
                    qT3[:, 2 * dp_:2 * dp_ + 2, t2 * NT:(t2 + 1) * NT], pq[:])

        kvs_cm.__exit__(None, None, None)
        pbN_cm.__exit__(None, None, None)
        pbps_cm.__exit__(None, None, None)
        pbA_cm.__exit__(None, None, None)

        # ---- more weights during the collective window ----
        hR_cm = tc.tile_pool(name="hpool", bufs=1, side="right")
        hRp = hR_cm.__enter__()
        hB = hRp.tile([P, CT, TQ], F32, tag="hB", name="hB")            # 32KB
        pe_cm = tc.tile_pool(name="pe", bufs=1, side="right")
        pep = pe_cm.__enter__()
        w13 = pep.tile([P, CT, FF], FP8, tag="w13", name="w13")         # 12KB
        nc.scalar.dma_start(w13[:, :, :],
                            w1[:, :].rearrange("(c p) n -> p c n", p=P))
        fB3 = pep.tile([P, CT, TQ], FP8, tag="fB3", name="fB3")         # 8KB
        wo_cm = tc.tile_pool(name="wopool", bufs=1, side="right")
        wop = wo_cm.__enter__()
        wo3 = wop.tile([P, CT, C], FP8, tag="wo3", name="wo3")
        nc.scalar.dma_start(wo3[:, :, :],
                            wo[:, :].rearrange("(c p) n -> p c n", p=P))

        # ---- gathered K/V reload: K first (scores need it), then V ----
        kT_cm = tc.tile_pool(name="kTpool", bufs=1)
        kTp = kT_cm.__enter__()
        kT3 = kTp.tile([P, DB, T], FP8, tag="kT", name="kT3")           # 32KB
        vB_cm = tc.tile_pool(name="vBpool", bufs=1)
        vBp = vB_cm.__enter__()
        vB3 = vBp.tile([P, TJ, C], FP8, tag="vB", name="vB3")           # 32KB

        kg_r = kg_d[:, :].rearrange("(r c p) t -> p r c t", p=P, r=4)
        for hp in range(H):
            for r in range(4):
                nc.sync.dma_start(
                    kT3[:, 2 * hp:2 * hp + 2, r * TQ:(r + 1) * TQ],
                    kg_r[:, r, 2 * hp:2 * hp + 2, :])
        vg_r = vg_d[:, :].rearrange("(r j p) c -> p r j c", p=P, r=4)
        for r in range(4):
            nc.sync.dma_start(vB3[:, r * (TQ // P):(r + 1) * (TQ // P), 0:NT],
                              vg_r[:, r, :, 0:NT])
        for r in range(4):
            nc.sync.dma_start(vB3[:, r * (TQ // P):(r + 1) * (TQ // P), NT:C],
                              vg_r[:, r, :, NT:C])

        # ---------------- attention (+ interleaved Wo/residual) ----------------
        pc_cm = tc.tile_pool(name="pc", bufs=1)
        pcp = pc_cm.__enter__()
        pss_cm = tc.tile_pool(name="ps_s", bufs=2, space="PSUM")
        pss = pss_cm.__enter__()
        pso_cm = tc.tile_pool(name="ps_o", bufs=1, space="PSUM")
        pso = pso_cm.__enter__()
        for ti in range(TQT):
            for h in range(H):
                et3 = pcp.tile([P, TJ, NT], FP8, tag="exp", bufs=2, name="et3")
                q_sl = qT3[:, 2 * h:2 * h + 2, ti * NT:(ti + 1) * NT]
                HN = NT // 2
                for jp in range(JP):
                    for qh in range(2):
                        psc = pss.tile([P, 2, HN], F32, tag="s", bufs=4,
                                       name="psc")
                        for half in range(2):
                            tj = 2 * jp + half
                            nc.tensor.matmul(
                                psc[:, half, :],
                                kT3[:, 2 * h:2 * h + 2, tj * P:(tj + 1) * P],
                                q_sl[:, :, qh * HN:(qh + 1) * HN],
                                start=True, stop=True, perf_mode=DRW)
                        e_sl = et3[:, 2 * jp:2 * jp + 2, qh * HN:(qh + 1) * HN]
                        if jp in SCH_SET or (jp == 6 and qh == 1):
                            sch = pcp.tile([P, 2, HN], I32, tag="sch", bufs=8,
                                           name="sch")
                            nc.vector.tensor_scalar(sch[:], psc[:],
                                                    SCH_A * EXP_SCALE, SCH_B,
                                                    ALU.mult, ALU.add)
                            nc.gpsimd.tensor_copy(e_sl, sch[:].bitcast(F32))
                        else:
                            nc.scalar.activation(e_sl, psc[:], AF.Exp,
                                                 scale=EXP_SCALE)
                po0 = pso.tile([P, NT], F32, tag="po0", name="po0")
                po1 = pso.tile([P, NT], F32, tag="po1", name="po1")
                pr = pso.tile([P, NT], F32, tag="pr", name="pr")
                for jp in range(JP):
                    e_sl = et3[:, 2 * jp:2 * jp + 2, :]
                    st_, sp_ = (jp == 0), (jp == JP - 1)
                    nc.tensor.matmul(pr[:], ones_d[:], e_sl, start=st_, stop=sp_,
                                     perf_mode=DRW, skip_group_check=True)
                    nc.tensor.matmul(po0[:],
                                     vB3[:, 2 * jp:2 * jp + 2, h * DH: h * DH + P],
                                     e_sl, start=st_, stop=sp_,
                                     perf_mode=DRW, skip_group_check=True)
                    nc.tensor.matmul(po1[:],
                                     vB3[:, 2 * jp:2 * jp + 2, h * DH + P:(h + 1) * DH],
                                     e_sl, start=st_, stop=sp_,
                                     perf_mode=DRW, skip_group_check=True)
                rec = pcp.tile([P, NT], F32, tag="rec", bufs=1, name="rec")
                nc.vector.reciprocal(rec[:], pr[:])
                nc.vector.tensor_mul(oT3[:, 2 * h, ti * NT:(ti + 1) * NT],
                                     po0[:], rec[:])
                nc.vector.tensor_mul(oT3[:, 2 * h + 1, ti * NT:(ti + 1) * NT],
                                     po1[:], rec[:])
            # ---- Wo + residual for this token half ----
            t2 = ti
            for cb in range(CT):
                if t2 == TQT - 1:
                    ph_tag = ("ph", "po0", "po1")[cb % 3]
                else:
                    ph_tag = "ph"
                ph = pso.tile([P, NT], F32, tag=ph_tag, bufs=1, name="ph")
                for cp_ in range(CT // 2):
                    nc.tensor.matmul(
                        ph[:],
                        wo3[:, 2 * cp_:2 * cp_ + 2, cb * P:(cb + 1) * P],
                        oT3[:, 2 * cp_:2 * cp_ + 2, t2 * NT:(t2 + 1) * NT],
                        start=(cp_ == 0), stop=(cp_ == CT // 2 - 1),
                        perf_mode=DRW)
                nc.vector.scalar_tensor_tensor(
                    hB[:, cb, t2 * NT:(t2 + 1) * NT], ph[:],
                    1.0 / (OSC * WS), xbT[:, cb, t2 * NT:(t2 + 1) * NT],
                    ALU.mult, ALU.add)
        pso_cm.__exit__(None, None, None)
        pss_cm.__exit__(None, None, None)
        pc_cm.__exit__(None, None, None)
        vB_cm.__exit__(None, None, None)
        kT_cm.__exit__(None, None, None)
        wo_cm.__exit__(None, None, None)

        # w23 loads into the space freed by the attention pools
        pf_cm = tc.tile_pool(name="pf", bufs=1)
        pfp = pf_cm.__enter__()
        w23 = pfp.tile([P, FFB, C], FP8, tag="w23", name="w23")         # 12KB
        gB3 = pfp.tile([P, FFB, TQ], FP8, tag="gB3", name="gB3")        # 12KB
        nc.sync.dma_start(w23[:, :, :],
                          w2[:, :].rearrange("(f p) n -> p f n", p=P))

        # ---------------- FFN ----------------
        peps_cm = tc.tile_pool(name="pe_ps", bufs=2, space="PSUM")
        peps = peps_cm.__enter__()
        for t2 in range(TQT):
            sq3 = pep.tile([P, CT, NT], FP8, tag="sqe", bufs=1, name="sqe")
            for cp_ in range(CT // 2):
                eng = nc.gpsimd if cp_ != 3 else nc.vector
                eng.tensor_mul(sq3[:, 2 * cp_:2 * cp_ + 2, :],
                               hB[:, 2 * cp_:2 * cp_ + 2, t2 * NT:(t2 + 1) * NT],
                               hB[:, 2 * cp_:2 * cp_ + 2, t2 * NT:(t2 + 1) * NT])
            ss = peps.tile([P, NT], F32, tag="sse", bufs=1, name="sse")
            for cp_ in range(CT // 2):
                nc.tensor.matmul(ss[:], ones8[:], sq3[:, 2 * cp_:2 * cp_ + 2, :],
                                 start=(cp_ == 0), stop=(cp_ == CT // 2 - 1),
                                 perf_mode=DRW)
            sqt = pep.tile([P, NT], F32, tag="sqte", bufs=1, name="sqte")
            nc.scalar.activation(sqt[:], ss[:], AF.Sqrt, scale=1.0 / C, bias=eps_t[:])
            rn = pep.tile([P, NT], F32, tag="rne", bufs=1, name="rne")
            nc.vector.reciprocal(rn[:], sqt[:])
            for ci in range(CT):
                eng = nc.gpsimd if ci % 8 < 5 else nc.vector
                eng.tensor_mul(fB3[:, ci, t2 * NT:(t2 + 1) * NT],
                               hB[:, ci, t2 * NT:(t2 + 1) * NT], rn[:])
        for t2 in range(TQT):
            for fp_ in range(FFB // 2):
                pu = peps.tile([P, 2, NT], F32, tag="pu", bufs=2, name="pu")
                for half in range(2):
                    fb = 2 * fp_ + half
                    for cp_ in range(CT // 2):
                        nc.tensor.matmul(
                            pu[:, half, :],
                            w13[:, 2 * cp_:2 * cp_ + 2, fb * P:(fb + 1) * P],
                            fB3[:, 2 * cp_:2 * cp_ + 2, t2 * NT:(t2 + 1) * NT],
                            start=(cp_ == 0), stop=(cp_ == CT // 2 - 1),
                            perf_mode=DRW)
                nc.scalar.activation(
                    gB3[:, 2 * fp_:2 * fp_ + 2, t2 * NT:(t2 + 1) * NT],
                    pu[:], AF.Gelu, scale=1.0 / WS)
        for t2 in range(TQT):
            yB = pfp.tile([P, CT, NT], F32, tag="yB", bufs=2, name="yB")
            for cb in range(CT):
                py = peps.tile([P, NT], F32, tag="py", bufs=3, name="py")
                for fp_ in range(FFB // 2):
                    nc.tensor.matmul(
                        py[:],
                        w23[:, 2 * fp_:2 * fp_ + 2, cb * P:(cb + 1) * P],
                        gB3[:, 2 * fp_:2 * fp_ + 2, t2 * NT:(t2 + 1) * NT],
                        start=(fp_ == 0), stop=(fp_ == FFB // 2 - 1),
                        perf_mode=DRW)
                nc.vector.scalar_tensor_tensor(
                    yB[:, cb, :], py[:], 1.0 / WS,
                    hB[:, cb, t2 * NT:(t2 + 1) * NT], ALU.mult, ALU.add)
            out_r = out[:, :].rearrange("(c p) t -> p c t", p=P)
            for cq_ in range(CT):
                nc.sync.dma_start(
                    out_r[:, cq_:cq_ + 1, t2 * NT:(t2 + 1) * NT],
                    yB[:, cq_:cq_ + 1, :])
        peps_cm.__exit__(None, None, None)
        pf_cm.__exit__(None, None, None)
        pe_cm.__exit__(None, None, None)
        hR_cm.__exit__(None, None, None)
        xb_cm.__exit__(None, None, None)
        qo_cm.__exit__(None, None, None)
        dram_cm.__exit__(None, None, None)
        cpool_cm.__exit__(None, None, None)

        sched_state, snap = tc.schedule_and_allocate()
        _CACHE["predicted_ns"] = snap.time if snap is not None else None
        try:
            _CACHE["dispatch_ns"] = sched_state.get_inst_dispatch_ns()
        except Exception:
            _CACHE["dispatch_ns"] = None

    nc.finalize()
    return nc


def get_nc():
    if "nc" not in _CACHE:
        _CACHE["nc"] = _build()
    return _CACHE["nc"]


def _prep_inputs(inputs):
    f8 = ml_dtypes.float8_e4m3
    x = np.asarray(inputs["x"], dtype=np.float32)
    g_attn = np.asarray(inputs["g_attn"], np.float32)
    g_ff = np.asarray(inputs["g_ff"], np.float32)
    wq8 = (g_attn[:, None] * np.asarray(inputs["Wq"], np.float32) * WS_QKV).astype(f8)
    wk8 = (g_attn[:, None] * np.asarray(inputs["Wk"], np.float32) * WS_QKV).astype(f8)
    wv8 = (g_attn[:, None] * np.asarray(inputs["Wv"], np.float32) * WS_QKV).astype(f8)
    wo8 = (np.asarray(inputs["Wo"], np.float32) * WS).astype(f8)
    w18 = (g_ff[:, None] * np.asarray(inputs["W1"], np.float32) * WS).astype(f8)
    w28 = (np.asarray(inputs["W2"], np.float32) * WS).astype(f8)
    in_maps = []
    for core in range(8):
        b, cq = divmod(core, 4)
        xc = np.ascontiguousarray(x[b][:, cq * TQ:(cq + 1) * TQ])
        in_maps.append({
            "xqb": xc.astype(ml_dtypes.bfloat16),
            "wq": wq8, "wk": wk8, "wv": wv8, "wo": wo8, "w1": w18, "w2": w28,
        })
    return in_maps


def run(inputs, **kwargs):
    nc = get_nc()
    in_maps = _prep_inputs(inputs)
    res = run_bass_kernel_spmd(nc, in_maps, core_ids=list(range(8)), **kwargs)
    out = np.empty((B, C, T), np.float32)
    for core in range(8):
        b, cq = divmod(core, 4)
        out[b][:, cq * TQ:(cq + 1) * TQ] = res.results[core]["out"]
    return out, res


def kernel(**inputs) -> np.ndarray:
    out, _ = run(inputs)
    return out
